# revision 19
# baseline (speedup 1.0000x reference)
"""GAT-style 2-layer knowledge-graph encoder on 8 trn2 NeuronCores.

Sharding: query rows, 512 per core. Scores are built transposed ([j, q]) so
the exp'd attention matrix is directly the matmul lhsT (no PE transposes).
The softmax denominator comes from a ones-column appended to the gathered
Wh payload (an extra matmul output column, no reduction pass). Wh for each
layer is computed on the owning shard and AllGathered on-device (bf16).

Host->device traffic is minimized (the wall clock is transfer-bound over
the axon tunnel, ~80 MB/s):
  - weights ship as 1/8 row-shards in one bf16 blob and are AllGathered
    on device (collectives can't read IO tensors, so shards are staged
    through internal DRAM first);
  - edge weights + mask are fused into ONE int8 array in natural row
    layout:  q[i,j] = round(127*ew) on edges (incl. self-loops), else
    -1 (or 0 when ew == 0 exactly).  A gpsimd casting DMA widens int8
    -> bf16; after a PE transpose the device derives
      ewp = max(q, 0)/127   (score multiplier) and
      mt  = (q >= 0)        (post-exp multiplier),
    which reproduces the reference's exp(0)=1 behaviour when ew == 0
    (uniform quantization tracks exp's absolute-argument sensitivity
    better than bf16's relative error);
  - node features ship as bf16 rows and are PE-transposed on device;
  - h2 returns as bf16 and is widened to f32 on the host.
"""

import numpy as np
import ml_dtypes

try:
    import jax
    jax.config.update("jax_compilation_cache_dir", "/tmp/jax_ccache")
    jax.config.update("jax_persistent_cache_min_compile_time_secs", 0)
    jax.config.update("jax_persistent_cache_min_entry_size_bytes", 0)
except Exception:
    pass

import concourse.bass as bass
import concourse.bacc as bacc
import concourse.mybir as mybir
from concourse import tile, masks
from concourse.bass_utils import run_bass_kernel_spmd
from concourse.alu_op_type import AluOpType as alu

BF16 = mybir.dt.bfloat16
F32 = mybir.dt.float32
I8 = mybir.dt.int8

P = 128
NCORES = 8
N = 4096
NSH = 512          # rows per core
H = 4
DIN = 768
HID = 512
F1 = 2048
DOUT = 768
C0 = 514           # 512 Wh + ones + pad  (bf16)
C1 = 770           # 768 Wh + ones + pad  (bf16)
ALPHA = 0.2
EPS = 1e-5
NIB = NSH // P     # 4 row-blocks per core
CH = 4             # j-tiles per chunk
NCHUNK = (N // P) // CH
NJT = N // P       # 32 j-tiles
AF = mybir.ActivationFunctionType

# parameter shard row counts (full rows / 8)
W0R = H * DIN // NCORES      # 384 rows of [., HID]
W1R = H * F1 // NCORES       # 1024 rows of [., DOUT]
RP0R = DIN // NCORES         # 96 rows of [., F1]
RP1R = F1 // NCORES          # 256 rows of [., DOUT]

# bf16 blob layout (element offsets): one transfer per core
NF_SZ = NSH * DIN
EWS_SZ = NSH * N
W0_SZ = W0R * HID
W1_SZ = W1R * DOUT
RP0_SZ = RP0R * F1
RP1_SZ = RP1R * DOUT
NF_OFF = 0
W0_OFF = NF_OFF + NF_SZ
W1_OFF = W0_OFF + W0_SZ
RP0_OFF = W1_OFF + W1_SZ
RP1_OFF = RP0_OFF + RP0_SZ
BLOB_SZ = RP1_OFF + RP1_SZ
QSCALE = 127.0

# f32 smalls blob layout
SB_A0 = 0
SB_A1 = SB_A0 + H * 2 * HID
SB_RP0B = SB_A1 + H * 2 * DOUT
SB_RP1B = SB_RP0B + F1
SB_LN0G = SB_RP1B + DOUT
SB_LN0B = SB_LN0G + F1
SB_LN1G = SB_LN0B + F1
SB_LN1B = SB_LN1G + DOUT
SBLOB_SZ = SB_LN1B + DOUT


def build_nc():
    nc = bacc.Bacc(num_devices=NCORES)

    blob = nc.declare_dram_parameter("blob", [1, BLOB_SZ], BF16,
                                     isOutput=False)
    qblob = nc.declare_dram_parameter("qblob", [1, EWS_SZ], I8,
                                      isOutput=False)
    sblob = nc.declare_dram_parameter("sblob", [1, SBLOB_SZ], F32,
                                      isOutput=False)
    h2 = nc.declare_dram_parameter("h2", [NSH, DOUT], BF16, isOutput=True)

    ewsTd = nc.dram_tensor("ewsTd", [N, NSH], BF16)
    W0i = nc.dram_tensor("W0i", [W0R, HID], BF16)
    W1i = nc.dram_tensor("W1i", [W1R, DOUT], BF16)
    rp0wi = nc.dram_tensor("rp0wi", [RP0R, F1], BF16)
    rp1wi = nc.dram_tensor("rp1wi", [RP1R, DOUT], BF16)
    gW0 = nc.dram_tensor("gW0", [H * DIN, HID], BF16)
    gW1 = nc.dram_tensor("gW1", [H * F1, DOUT], BF16)
    grp0w = nc.dram_tensor("grp0w", [DIN, F1], BF16)
    grp1w = nc.dram_tensor("grp1w", [F1, DOUT], BF16)

    g0_in = nc.dram_tensor("g0_in", [H, NSH, C0], BF16)
    g0_out = nc.dram_tensor("g0_out", [NCORES, H, NSH, C0], BF16)
    g0s_in = nc.dram_tensor("g0s_in", [H, NSH, 2], F32)
    g0s_out = nc.dram_tensor("g0s_out", [NCORES, H, NSH, 2], F32)
    g1_in = nc.dram_tensor("g1_in", [H, NSH, C1], BF16)
    g1_out = nc.dram_tensor("g1_out", [NCORES, H, NSH, C1], BF16)
    g1s_in = nc.dram_tensor("g1s_in", [H, NSH, 2], F32)
    g1s_out = nc.dram_tensor("g1s_out", [NCORES, H, NSH, 2], F32)

    groups = [list(range(NCORES))]

    with tile.TileContext(nc) as tc:
        with (
            tc.tile_pool(name="persist", bufs=1) as pp,
            tc.tile_pool(name="sb", bufs=2) as sb,
            tc.tile_pool(name="small", bufs=3) as sm,
        ):
            ident = pp.tile([P, P], F32)
            masks.make_identity(nc, ident[:])
            identb = pp.tile([P, P], BF16)
            nc.vector.tensor_copy(identb[:], ident[:])
            h2pre = pp.tile([P, NIB, DOUT], F32)

            # param shards -> full weights, gathered on-device.
            # (collectives cannot read IO tensors; stage via internal DRAM)
            nc.sync.dma_start(out=W0i[:, :],
                              in_=blob[0, W0_OFF:W0_OFF + W0_SZ])
            nc.sync.dma_start(out=rp0wi[:, :],
                              in_=blob[0, RP0_OFF:RP0_OFF + RP0_SZ])
            nc.sync.dma_start(out=W1i[:, :],
                              in_=blob[0, W1_OFF:W1_OFF + W1_SZ])
            nc.sync.dma_start(out=rp1wi[:, :],
                              in_=blob[0, RP1_OFF:RP1_OFF + RP1_SZ])
            nc.gpsimd.collective_compute(
                "AllGather", alu.bypass, replica_groups=groups,
                ins=[W0i[:, :].opt()], outs=[gW0[:, :].opt()])
            nc.gpsimd.collective_compute(
                "AllGather", alu.bypass, replica_groups=groups,
                ins=[rp0wi[:, :].opt()], outs=[grp0w[:, :].opt()])
            nc.gpsimd.collective_compute(
                "AllGather", alu.bypass, replica_groups=groups,
                ins=[W1i[:, :].opt()], outs=[gW1[:, :].opt()])
            nc.gpsimd.collective_compute(
                "AllGather", alu.bypass, replica_groups=groups,
                ins=[rp1wi[:, :].opt()], outs=[grp1w[:, :].opt()])

            def bcast(pool, dram_row, width, name):
                row = pool.tile([1, width], F32, tag="bc_row", bufs=1,
                                name=f"r_{name}")
                nc.sync.dma_start(out=row[:], in_=dram_row)
                out = pool.tile([P, width], F32, name=f"b_{name}")
                nc.gpsimd.partition_broadcast(out[:], row[0:1, :])
                return out

            def ln_elu(pool, x_ap, gb, bb, width, out_ap, do_elu):
                """LN over free dim; x_ap is clobbered as scratch (B0)."""
                b1 = pool.tile([P, width], F32, tag="ln_b1", bufs=1,
                               name="ln_b1")
                b2 = pool.tile([P, width], F32, tag="ln_b2", bufs=1,
                               name="ln_b2")
                s1 = sm.tile([P, 1], F32, tag="ln_s1", name="ln_s1")
                nc.vector.tensor_reduce(s1[:], x_ap, mybir.AxisListType.X,
                                        alu.add)
                negmean = sm.tile([P, 1], F32, tag="ln_nm", name="ln_nm")
                nc.vector.tensor_single_scalar(negmean[:], s1[:],
                                               -1.0 / width, alu.mult)
                nc.scalar.activation(b1[:], x_ap, AF.Identity,
                                     bias=negmean[:, 0:1])          # t
                ssq = sm.tile([P, 1], F32, tag="ln_ssq", name="ln_ssq")
                nc.scalar.activation(b2[:], b1[:], AF.Square,
                                     accum_out=ssq[:, 0:1])
                var = sm.tile([P, 1], F32, tag="ln_var", name="ln_var")
                nc.vector.tensor_scalar(var[:], ssq[:], 1.0 / width, EPS,
                                        alu.mult, alu.add)
                std = sm.tile([P, 1], F32, tag="ln_std", name="ln_std")
                nc.scalar.activation(std[:], var[:], AF.Sqrt)
                rstd = sm.tile([P, 1], F32, tag="ln_rstd", name="ln_rstd")
                nc.vector.reciprocal(rstd[:], std[:])
                nc.scalar.mul(b2[:], b1[:], rstd[:, 0:1])           # u
                nc.vector.tensor_tensor(b1[:], b2[:], gb, alu.mult)  # v
                if not do_elu:
                    nc.vector.tensor_tensor(out_ap, b1[:], bb, alu.add)
                    return
                nc.vector.tensor_tensor(b2[:], b1[:], bb, alu.add)   # w
                nc.vector.tensor_single_scalar(b1[:], b2[:], 0.0, alu.min)
                nc.scalar.activation(x_ap, b1[:], AF.Exp)            # -> B0
                nc.vector.tensor_single_scalar(b1[:], b2[:], 0.0, alu.max)
                nc.vector.scalar_tensor_tensor(out_ap, x_ap, -1.0, b1[:],
                                               alu.add, alu.add)

            def attention(lid, O, N1, g_out, gs_out, gs_in, dest, mean_heads):
                CX = O + 2
                with (
                    tc.tile_pool(name=f"att{lid}", bufs=1) as ap_,
                    tc.tile_pool(name=f"att{lid}_d", bufs=3) as ad,
                    tc.tile_pool(name=f"att{lid}_ps", bufs=1,
                                 space="PSUM") as aps,
                ):
                    ssb = []
                    for h in range(H):
                        row = sm.tile([1, NSH], F32, tag="ssrow",
                                      name=f"ssrow{lid}_{h}")
                        nc.sync.dma_start(
                            out=row[:],
                            in_=gs_in[h, :, 0:1].rearrange("q c -> c q"))
                        sbh = ap_.tile([P, NSH], F32, name=f"ssb{lid}_{h}")
                        nc.gpsimd.partition_broadcast(sbh[:], row[0:1, :])
                        ssb.append(sbh)
                    acc = [ap_.tile([P, NIB, O + 1], F32,
                                    name=f"acc{lid}_{hh}") for hh in range(H)]
                    whs = ap_.tile([P, CH, H, CX], BF16)
                    svs = ap_.tile([P, CH, H, 2], F32)
                    ewsTv = ewsTd.rearrange("(c p) q -> p c q", p=P)
                    for jc in range(NCHUNK):
                        ewsc = ad.tile([P, CH, NSH], BF16, tag="ewsc",
                                       bufs=1, name="ewsc")
                        nc.sync.dma_start(
                            out=ewsc[:],
                            in_=ewsTv[:, jc * CH:(jc + 1) * CH, :])
                        ewpc = ad.tile([P, CH, NSH], BF16, tag="ewpc",
                                       bufs=1, name="ewpc")
                        nc.vector.tensor_scalar(
                            ewpc[:], ewsc[:], 0.0, 1.0 / QSCALE,
                            alu.max, alu.mult)
                        mtc = ad.tile([P, CH, NSH], BF16, tag="mtc",
                                      bufs=1, name="mtc")
                        nc.vector.tensor_single_scalar(
                            mtc[:], ewsc[:], 0.0, alu.is_ge)
                        for jt in range(CH):
                            jg = jc * CH + jt
                            s, r = jg // NIB, jg % NIB
                            nc.sync.dma_start(
                                out=whs[:, jt, :, :],
                                in_=g_out[s, :, r * P:(r + 1) * P, :]
                                .rearrange("h p c -> p h c"))
                            nc.sync.dma_start(
                                out=svs[:, jt, :, :],
                                in_=gs_out[s, :, r * P:(r + 1) * P, :]
                                .rearrange("h p c -> p h c"))
                        for h in range(H):
                            psa = [aps.tile([P, N1], F32, tag=f"psa{qb}",
                                            name=f"psa_{qb}")
                                   for qb in range(NIB)]
                            psb = [aps.tile([P, 257], F32, tag=f"psb{qb}",
                                            name=f"psb_{qb}")
                                   for qb in range(NIB)]
                            for jt in range(CH):
                                e = ad.tile([P, NSH], F32, tag="e", name="e")
                                nc.scalar.activation(
                                    e[:], ssb[h][:, :], AF.Lrelu,
                                    bias=svs[:, jt, h, 1:2], alpha=ALPHA)
                                att = ad.tile([P, NSH], F32, tag="att",
                                              name="att")
                                nc.vector.tensor_tensor(
                                    att[:], e[:], ewpc[:, jt, :], alu.mult)
                                pt = ad.tile([P, NSH], BF16, tag="pt",
                                             name="pt")
                                nc.scalar.activation(pt[:], att[:], AF.Exp)
                                ptm = ad.tile([P, NSH], BF16, tag="ptm",
                                              name="ptm")
                                nc.vector.tensor_tensor(
                                    ptm[:], pt[:], mtc[:, jt, :], alu.mult)
                                for qb in range(NIB):
                                    lhs = ptm[:, qb * P:(qb + 1) * P]
                                    nc.tensor.matmul(
                                        psa[qb][:], lhs, whs[:, jt, h, 0:N1],
                                        start=(jt == 0), stop=(jt == CH - 1))
                                    nc.tensor.matmul(
                                        psb[qb][:], lhs,
                                        whs[:, jt, h, N1:N1 + 257],
                                        start=(jt == 0), stop=(jt == CH - 1))
                            for qb in range(NIB):
                                if jc == 0:
                                    nc.vector.tensor_copy(
                                        acc[h][:, qb, 0:N1], psa[qb][:])
                                    nc.vector.tensor_copy(
                                        acc[h][:, qb, N1:O + 1], psb[qb][:])
                                else:
                                    nc.vector.scalar_tensor_tensor(
                                        acc[h][:, qb, 0:N1], psa[qb][:], 0.0,
                                        acc[h][:, qb, 0:N1], alu.add, alu.add)
                                    nc.vector.scalar_tensor_tensor(
                                        acc[h][:, qb, N1:O + 1], psb[qb][:],
                                        0.0, acc[h][:, qb, N1:O + 1],
                                        alu.add, alu.add)
                    for h in range(H):
                        for qb in range(NIB):
                            den = sm.tile([P, 1], F32, tag="den", name="den")
                            if mean_heads:
                                nc.vector.tensor_single_scalar(
                                    den[:], acc[h][:, qb, O:O + 1], float(H),
                                    alu.mult)
                            else:
                                nc.vector.tensor_copy(
                                    den[:], acc[h][:, qb, O:O + 1])
                            rcp = sm.tile([P, 1], F32, tag="rcp", name="rcp")
                            nc.vector.reciprocal(rcp[:], den[:])
                            out_ap = (dest[:, qb, 0:O] if mean_heads else
                                      dest[:, qb, h * O:(h + 1) * O])
                            nc.vector.scalar_tensor_tensor(
                                out_ap, acc[h][:, qb, 0:O], rcp[:, 0:1],
                                out_ap, alu.mult, alu.add)

            # ---- poolX: h1pre / h1 / h1T ----
            with tc.tile_pool(name="poolX", bufs=1) as px:
                # ===== prep: transpose ews on device -> ewsTd (DRAM) =====
                with (
                    tc.tile_pool(name="prep", bufs=2) as pr,
                    tc.tile_pool(name="prep_ps", bufs=2, space="PSUM") as prps,
                ):
                    ewsTw = ewsTd.rearrange("(jt p) q -> p jt q", p=P)
                    for qb in range(NIB):
                        ewq = pr.tile([P, N], BF16, tag="ewq", name="ewq")
                        nc.gpsimd.dma_start(
                            out=ewq[:],
                            in_=qblob[0, qb * P * N:(qb + 1) * P * N]
                            .rearrange("(p j) -> p j", p=P))
                        st = pr.tile([P, NJT, P], BF16, tag="ewst",
                                     name="ewst")
                        for jt in range(NJT):
                            pstb = prps.tile([P, P], BF16, tag="prtb",
                                             name="prtb")
                            nc.tensor.transpose(
                                pstb[:], ewq[:, jt * P:(jt + 1) * P],
                                identb[:])
                            nc.scalar.copy(st[:, jt, :], pstb[:])
                        nc.sync.dma_start(
                            out=ewsTw[:, :, qb * P:(qb + 1) * P],
                            in_=st[:])

                h1pre = px.tile([P, NIB, F1], F32)

                # ===== Phase A =====
                with (
                    tc.tile_pool(name="phA", bufs=1) as pa,
                    tc.tile_pool(name="phA_ps", bufs=2, space="PSUM") as paps,
                ):
                    a0b = bcast(pa, sblob[:, SB_A0:SB_A0 + H * 2 * HID],
                                H * 2 * HID, "a0")
                    a0b = a0b.rearrange("p (h c) -> p h c", h=H)
                    rp0bb = bcast(pa, sblob[:, SB_RP0B:SB_RP0B + F1],
                                  F1, "rp0b")
                    s_sb0 = pa.tile([P, H, NIB, 2], F32)

                    # transpose nf on device -> nfTbf
                    nfTbf = pa.tile([P, DIN // P, NSH], BF16)
                    with (
                        tc.tile_pool(name="nfp", bufs=1) as npr,
                        tc.tile_pool(name="nfp_ps", bufs=2,
                                     space="PSUM") as nps,
                    ):
                        nfsb = npr.tile([P, NIB, DIN], BF16)
                        nc.sync.dma_start(
                            out=nfsb[:],
                            in_=blob[0, NF_OFF:NF_OFF + NF_SZ]
                            .rearrange("(b p f) -> p b f", p=P, f=DIN))
                        for qb in range(NIB):
                            for kb in range(DIN // P):
                                pst = nps.tile([P, P], BF16, tag="nft",
                                               name="nft")
                                nc.tensor.transpose(
                                    pst[:],
                                    nfsb[:, qb, kb * P:(kb + 1) * P],
                                    identb[:])
                                nc.scalar.copy(
                                    nfTbf[:, kb, qb * P:(qb + 1) * P],
                                    pst[:])

                    for h in range(H):
                        psv = [paps.tile([P, HID], F32, tag=f"wh0ps{ib}",
                                         bufs=1, name=f"wh0ps_{ib}")
                               for ib in range(NIB)]
                        for k in range(DIN // P):
                            w0t = sb.tile([P, HID], BF16, tag="w0t",
                                          bufs=3, name="w0t")
                            nc.sync.dma_start(
                                out=w0t[:],
                                in_=gW0[h * DIN + k * P:h * DIN + (k + 1) * P,
                                        :])
                            for ib in range(NIB):
                                nc.tensor.matmul(
                                    psv[ib][:],
                                    nfTbf[:, k, ib * P:(ib + 1) * P],
                                    w0t[:],
                                    start=(k == 0), stop=(k == DIN // P - 1))
                        for ib in range(NIB):
                            ps = psv[ib]
                            whtmp = sb.tile([P, HID], F32, tag="whtmp",
                                            bufs=1, name="whtmp")
                            nc.scalar.copy(whtmp[:], ps[:])
                            for which in range(2):
                                tmp = sb.tile([P, HID], F32, tag="sred",
                                              bufs=1, name="sred")
                                nc.vector.tensor_tensor(
                                    tmp[:], whtmp[:],
                                    a0b[:, h, which * HID:(which + 1) * HID],
                                    alu.mult)
                                nc.vector.tensor_reduce(
                                    s_sb0[:, h, ib, which:which + 1], tmp[:],
                                    mybir.AxisListType.X, alu.add)
                            pack = sb.tile([P, C0], BF16, tag="pack0",
                                           name="pack")
                            nc.vector.tensor_copy(pack[:, 0:HID], whtmp[:])
                            nc.vector.memset(pack[:, HID:HID + 1], 1.0)
                            nc.vector.memset(pack[:, HID + 1:C0], 0.0)
                            nc.sync.dma_start(
                                out=g0_in[h, ib * P:(ib + 1) * P, :],
                                in_=pack[:])
                    nc.sync.dma_start(
                        out=g0s_in.rearrange("h (ib p) c -> p h ib c", p=P),
                        in_=s_sb0[:])
                    nc.gpsimd.collective_compute(
                        "AllGather", alu.bypass, replica_groups=groups,
                        ins=[g0_in[:, :, :].opt()],
                        outs=[g0_out[:, :, :, :].opt()])
                    nc.gpsimd.collective_compute(
                        "AllGather", alu.bypass, replica_groups=groups,
                        ins=[g0s_in[:, :, :].opt()],
                        outs=[g0s_out[:, :, :, :].opt()])

                    rp0wsb = pa.tile([P, DIN // P, F1], BF16)
                    nc.sync.dma_start(
                        out=rp0wsb[:],
                        in_=grp0w.rearrange("(k p) o -> p k o", p=P))
                    for ib in range(NIB):
                        for oc in range(4):
                            ps2 = paps.tile([P, 512], F32, tag="rp0ps",
                                            name="ps2")
                            for k in range(DIN // P):
                                nc.tensor.matmul(
                                    ps2[:], nfTbf[:, k, ib * P:(ib + 1) * P],
                                    rp0wsb[:, k, oc * 512:(oc + 1) * 512],
                                    start=(k == 0), stop=(k == DIN // P - 1))
                            nc.vector.tensor_tensor(
                                h1pre[:, ib, oc * 512:(oc + 1) * 512],
                                ps2[:], rp0bb[:, oc * 512:(oc + 1) * 512],
                                alu.add)

                attention(0, HID, 256, g0_out, g0s_out, g0s_in, h1pre, False)

                h1T = px.tile([P, F1 // P, NSH], BF16)
                # ===== LN0 + ELU -> h1, transpose -> h1T =====
                with tc.tile_pool(name="ln0p", bufs=1) as lp0:
                    ln0gb = bcast(lp0, sblob[:, SB_LN0G:SB_LN0G + F1],
                                  F1, "ln0g")
                    ln0bb = bcast(lp0, sblob[:, SB_LN0B:SB_LN0B + F1],
                                  F1, "ln0b")
                    for ib in range(NIB):
                        ln_elu(lp0, h1pre[:, ib, :], ln0gb[:, :],
                               ln0bb[:, :], F1, h1pre[:, ib, :], True)
                with tc.tile_pool(name="trps", bufs=2, space="PSUM") as tps:
                    for ib in range(NIB):
                        for fb in range(F1 // P):
                            pst = tps.tile([P, P], F32, tag="pst",
                                           name="pst")
                            nc.tensor.transpose(
                                pst[:], h1pre[:, ib, fb * P:(fb + 1) * P],
                                ident[:])
                            nc.scalar.copy(
                                h1T[:, fb, ib * P:(ib + 1) * P], pst[:])

                # ===== Phase B =====
                with (
                    tc.tile_pool(name="phB", bufs=1) as pb,
                    tc.tile_pool(name="phB_d", bufs=3) as pbd,
                    tc.tile_pool(name="phB_ps", bufs=1, space="PSUM") as pbps,
                ):
                    a1bs = [bcast(pb, sblob[:, SB_A1 + hh * 2 * DOUT:
                                            SB_A1 + (hh + 1) * 2 * DOUT],
                                  2 * DOUT, f"a1_{hh}") for hh in range(H)]
                    rp1bb = bcast(pb, sblob[:, SB_RP1B:SB_RP1B + DOUT],
                                  DOUT, "rp1b")
                    s_sb1 = pb.tile([P, H, NIB, 2], F32)
                    halves = ((0, 512), (512, DOUT))
                    for h in range(H):
                        psw = [pbps.tile([P, DOUT], F32, tag=f"wh1ps{ib}",
                                         name=f"wh1ps_{ib}")
                               for ib in range(NIB)]
                        for k in range(F1 // P):
                            w1t = pbd.tile([P, DOUT], BF16, tag="w1t",
                                           name="w1t")
                            nc.sync.dma_start(
                                out=w1t[:],
                                in_=gW1[h * F1 + k * P:h * F1 + (k + 1) * P,
                                        :])
                            for ib in range(NIB):
                                for (o0, o1) in halves:
                                    nc.tensor.matmul(
                                        psw[ib][:, o0:o1],
                                        h1T[:, k, ib * P:(ib + 1) * P],
                                        w1t[:, o0:o1],
                                        start=(k == 0),
                                        stop=(k == F1 // P - 1))
                        for ib in range(NIB):
                            whtmp1 = sb.tile([P, DOUT], F32, tag="whtmp1",
                                             bufs=1, name="whtmp1")
                            nc.scalar.copy(whtmp1[:], psw[ib][:])
                            for which in range(2):
                                tmp = sb.tile([P, DOUT], F32, tag="sred1",
                                              bufs=1, name="tmp")
                                nc.vector.tensor_tensor(
                                    tmp[:], whtmp1[:],
                                    a1bs[h][:, which * DOUT:(which + 1) * DOUT],
                                    alu.mult)
                                nc.vector.tensor_reduce(
                                    s_sb1[:, h, ib, which:which + 1], tmp[:],
                                    mybir.AxisListType.X, alu.add)
                            pack1 = sb.tile([P, C1], BF16, tag="pack1",
                                            name="pack1")
                            nc.vector.tensor_copy(pack1[:, 0:DOUT],
                                                  whtmp1[:])
                            nc.vector.memset(pack1[:, DOUT:DOUT + 1], 1.0)
                            nc.vector.memset(pack1[:, DOUT + 1:C1], 0.0)
                            nc.sync.dma_start(
                                out=g1_in[h, ib * P:(ib + 1) * P, :],
                                in_=pack1[:])
                    nc.sync.dma_start(
                        out=g1s_in.rearrange("h (ib p) c -> p h ib c", p=P),
                        in_=s_sb1[:])
                    nc.gpsimd.collective_compute(
                        "AllGather", alu.bypass, replica_groups=groups,
                        ins=[g1_in[:, :, :].opt()],
                        outs=[g1_out[:, :, :, :].opt()])
                    nc.gpsimd.collective_compute(
                        "AllGather", alu.bypass, replica_groups=groups,
                        ins=[g1s_in[:, :, :].opt()],
                        outs=[g1s_out[:, :, :, :].opt()])

                    psr = [pbps.tile([P, DOUT], F32, tag=f"wh1ps{ib}",
                                     name=f"rp1ps_{ib}")
                           for ib in range(NIB)]
                    for k in range(F1 // P):
                        r1t = pbd.tile([P, DOUT], BF16, tag="r1t",
                                       name="r1t")
                        nc.sync.dma_start(
                            out=r1t[:], in_=grp1w[k * P:(k + 1) * P, :])
                        for ib in range(NIB):
                            for (o0, o1) in halves:
                                nc.tensor.matmul(
                                    psr[ib][:, o0:o1],
                                    h1T[:, k, ib * P:(ib + 1) * P],
                                    r1t[:, o0:o1],
                                    start=(k == 0), stop=(k == F1 // P - 1))
                    for ib in range(NIB):
                        nc.vector.tensor_tensor(
                            h2pre[:, ib, :], psr[ib][:], rp1bb[:, :],
                            alu.add)

            attention(1, DOUT, 512, g1_out, g1s_out, g1s_in, h2pre, True)

            # ===== LN1 -> h2 out =====
            with tc.tile_pool(name="ln1p", bufs=1) as lp1:
                ln1gb = bcast(lp1, sblob[:, SB_LN1G:SB_LN1G + DOUT],
                              DOUT, "ln1g")
                ln1bb = bcast(lp1, sblob[:, SB_LN1B:SB_LN1B + DOUT],
                              DOUT, "ln1b")
                for ib in range(NIB):
                    o = sb.tile([P, DOUT], BF16, tag="hout", name="o")
                    ln_elu(lp1, h2pre[:, ib, :], ln1gb[:, :], ln1bb[:, :],
                           DOUT, o[:], False)
                    nc.sync.dma_start(out=h2[ib * P:(ib + 1) * P, :],
                                      in_=o[:])

    nc.finalize()
    return nc


_NC_CACHE = None


def _get_nc():
    global _NC_CACHE
    if _NC_CACHE is None:
        _NC_CACHE = build_nc()
    return _NC_CACHE


_SCRATCH = {}


def _scratch(name, shape, dtype):
    buf = _SCRATCH.get(name)
    if buf is None or buf.shape != shape or buf.dtype != dtype:
        buf = np.empty(shape, dtype)
        _SCRATCH[name] = buf
    return buf


def build_in_maps(node_features, adjacency, edge_weights, W0, a0, W1, a1,
                  rp0_w, rp0_b, rp1_w, rp1_b, ln0_g, ln0_b, ln1_g, ln1_b):
    bf = ml_dtypes.bfloat16
    nf = np.asarray(node_features, np.float32).astype(bf)
    adj = np.asarray(adjacency)
    ew = np.asarray(edge_weights, np.float32)

    # q[i,j] = round(ew*127) on edges (incl. diagonal); -1 off edges
    # (0 if ew == 0 exactly, preserving the reference's exp(0)=1 quirk).
    conn = adj != 0
    np.fill_diagonal(conn, True)
    fbuf = _scratch("fbuf", (N, N), np.float32)
    np.multiply(ew, np.float32(QSCALE), out=fbuf)
    np.add(fbuf, np.float32(0.5), out=fbuf)
    q = np.where(conn, fbuf.astype(np.int8), -(ew != 0).astype(np.int8))

    w0 = np.asarray(W0, np.float32).reshape(H * DIN, HID).astype(bf)
    w1 = np.asarray(W1, np.float32).reshape(H * F1, DOUT).astype(bf)
    rp0w = np.asarray(rp0_w, np.float32).astype(bf)
    rp1w = np.asarray(rp1_w, np.float32).astype(bf)

    blob = _scratch("blob", (NCORES, BLOB_SZ), bf)
    for c in range(NCORES):
        rows = slice(c * NSH, (c + 1) * NSH)
        blob[c, NF_OFF:NF_OFF + NF_SZ] = nf[rows].reshape(-1)
        blob[c, W0_OFF:W0_OFF + W0_SZ] = \
            w0[c * W0R:(c + 1) * W0R].reshape(-1)
        blob[c, W1_OFF:W1_OFF + W1_SZ] = \
            w1[c * W1R:(c + 1) * W1R].reshape(-1)
        blob[c, RP0_OFF:RP0_OFF + RP0_SZ] = \
            rp0w[c * RP0R:(c + 1) * RP0R].reshape(-1)
        blob[c, RP1_OFF:RP1_OFF + RP1_SZ] = \
            rp1w[c * RP1R:(c + 1) * RP1R].reshape(-1)

    sblob = np.concatenate([
        np.asarray(a0, np.float32).reshape(-1),
        np.asarray(a1, np.float32).reshape(-1),
        np.asarray(rp0_b, np.float32).reshape(-1),
        np.asarray(rp1_b, np.float32).reshape(-1),
        np.asarray(ln0_g, np.float32).reshape(-1),
        np.asarray(ln0_b, np.float32).reshape(-1),
        np.asarray(ln1_g, np.float32).reshape(-1),
        np.asarray(ln1_b, np.float32).reshape(-1),
    ]).reshape(1, SBLOB_SZ)

    qb = q.reshape(NCORES, 1, EWS_SZ)
    return [{"blob": blob[c:c + 1], "qblob": qb[c], "sblob": sblob}
            for c in range(NCORES)]


def kernel(**inputs):
    in_maps = build_in_maps(**inputs)
    nc = _get_nc()
    res = run_bass_kernel_spmd(nc, in_maps, list(range(NCORES)))
    return np.concatenate([res.results[c]["h2"] for c in range(NCORES)],
                          axis=0).astype(np.float32)


# revision 23
# speedup vs baseline: 1.0266x; 1.0266x over previous
"""GAT-style 2-layer knowledge-graph encoder on 8 trn2 NeuronCores.

Sharding: query rows, 512 per core. Scores are built transposed ([j, q]) so
the exp'd attention matrix is directly the matmul lhsT (no PE transposes).
The softmax denominator comes from a ones-column appended to the gathered
Wh payload (an extra matmul output column, no reduction pass). Wh for each
layer is computed on the owning shard and AllGathered on-device (bf16).

Host->device traffic is minimized (the wall clock is transfer-bound over
the axon tunnel, ~80 MB/s):
  - weights ship as 1/8 row-shards in one bf16 blob and are AllGathered
    on device (collectives can't read IO tensors, so shards are staged
    through internal DRAM first);
  - edge weights + mask are fused into ONE int8 array in natural row
    layout:  q[i,j] = round(127*ew) on edges (incl. self-loops), else
    -1 (or 0 when ew == 0 exactly).  A gpsimd casting DMA widens int8
    -> bf16; after a PE transpose the device derives
      ewp = max(q, 0)/127   (score multiplier) and
      mt  = (q >= 0)        (post-exp multiplier),
    which reproduces the reference's exp(0)=1 behaviour when ew == 0
    (uniform quantization tracks exp's absolute-argument sensitivity
    better than bf16's relative error);
  - node features ship as bf16 rows and are PE-transposed on device;
  - h2 returns as bf16 and is widened to f32 on the host.
"""

import numpy as np
import ml_dtypes

try:
    import jax
    jax.config.update("jax_compilation_cache_dir", "/tmp/jax_ccache")
    jax.config.update("jax_persistent_cache_min_compile_time_secs", 0)
    jax.config.update("jax_persistent_cache_min_entry_size_bytes", 0)
except Exception:
    pass

import concourse.bass as bass
import concourse.bacc as bacc
import concourse.mybir as mybir
from concourse import tile, masks
from concourse.bass_utils import run_bass_kernel_spmd
from concourse.alu_op_type import AluOpType as alu

BF16 = mybir.dt.bfloat16
F32 = mybir.dt.float32
I8 = mybir.dt.int8

P = 128
NCORES = 8
N = 4096
NSH = 512          # rows per core
H = 4
DIN = 768
HID = 512
F1 = 2048
DOUT = 768
C0 = 514           # 512 Wh + ones + pad  (bf16)
C1 = 770           # 768 Wh + ones + pad  (bf16)
ALPHA = 0.2
EPS = 1e-5
NIB = NSH // P     # 4 row-blocks per core
CH = 4             # j-tiles per chunk
NCHUNK = (N // P) // CH
NJT = N // P       # 32 j-tiles
AF = mybir.ActivationFunctionType

# parameter shard row counts (full rows / 8)
W0R = H * DIN // NCORES      # 384 rows of [., HID]
W1R = H * F1 // NCORES       # 1024 rows of [., DOUT]
RP0R = DIN // NCORES         # 96 rows of [., F1]
RP1R = F1 // NCORES          # 256 rows of [., DOUT]

NF_SZ = NSH * DIN
EWS_SZ = NSH * N
W0_SZ = W0R * HID
W1_SZ = W1R * DOUT
RP0_SZ = RP0R * F1
RP1_SZ = RP1R * DOUT
QSCALE = 127.0

# int8 qblob layout (element offsets): ews rows + weight shards
QEWS_OFF = 0
QW0_OFF = QEWS_OFF + EWS_SZ
QW1_OFF = QW0_OFF + W0_SZ
QRP0_OFF = QW1_OFF + W1_SZ
QRP1_OFF = QRP0_OFF + RP0_SZ
QBLOB_SZ = QRP1_OFF + RP1_SZ

# bf16 blob layout: nf rows + FULL per-row dequant scales (replicated)
SC_W0 = H * DIN        # 3072 rows
SC_W1 = H * F1         # 8192 rows
SC_RP0 = DIN           # 768 rows
SC_RP1 = F1            # 2048 rows
NF_OFF = 0
SW0_OFF = NF_OFF + NF_SZ
SW1_OFF = SW0_OFF + SC_W0
SRP0_OFF = SW1_OFF + SC_W1
SRP1_OFF = SRP0_OFF + SC_RP0
BLOB_SZ = SRP1_OFF + SC_RP1

# f32 smalls blob layout
SB_A0 = 0
SB_A1 = SB_A0 + H * 2 * HID
SB_RP0B = SB_A1 + H * 2 * DOUT
SB_RP1B = SB_RP0B + F1
SB_LN0G = SB_RP1B + DOUT
SB_LN0B = SB_LN0G + F1
SB_LN1G = SB_LN0B + F1
SB_LN1B = SB_LN1G + DOUT
SBLOB_SZ = SB_LN1B + DOUT


def build_nc():
    nc = bacc.Bacc(num_devices=NCORES)

    blob = nc.declare_dram_parameter("blob", [1, BLOB_SZ], BF16,
                                     isOutput=False)
    qblob = nc.declare_dram_parameter("qblob", [1, QBLOB_SZ], I8,
                                      isOutput=False)
    sblob = nc.declare_dram_parameter("sblob", [1, SBLOB_SZ], F32,
                                      isOutput=False)
    h2 = nc.declare_dram_parameter("h2", [NSH, DOUT], BF16, isOutput=True)

    ewsTd = nc.dram_tensor("ewsTd", [N, NSH], BF16)
    W0i = nc.dram_tensor("W0i", [W0R, HID], I8)
    W1i = nc.dram_tensor("W1i", [W1R, DOUT], I8)
    rp0wi = nc.dram_tensor("rp0wi", [RP0R, F1], I8)
    rp1wi = nc.dram_tensor("rp1wi", [RP1R, DOUT], I8)
    gW0 = nc.dram_tensor("gW0", [H * DIN, HID], I8)
    gW1 = nc.dram_tensor("gW1", [H * F1, DOUT], I8)
    grp0w = nc.dram_tensor("grp0w", [DIN, F1], I8)
    grp1w = nc.dram_tensor("grp1w", [F1, DOUT], I8)

    g0_in = nc.dram_tensor("g0_in", [H, NSH, C0], BF16)
    g0_out = nc.dram_tensor("g0_out", [NCORES, H, NSH, C0], BF16)
    g0s_in = nc.dram_tensor("g0s_in", [H, NSH, 2], F32)
    g0s_out = nc.dram_tensor("g0s_out", [NCORES, H, NSH, 2], F32)
    g1_in = nc.dram_tensor("g1_in", [H, NSH, C1], BF16)
    g1_out = nc.dram_tensor("g1_out", [NCORES, H, NSH, C1], BF16)
    g1s_in = nc.dram_tensor("g1s_in", [H, NSH, 2], F32)
    g1s_out = nc.dram_tensor("g1s_out", [NCORES, H, NSH, 2], F32)

    groups = [list(range(NCORES))]

    with tile.TileContext(nc) as tc:
        with (
            tc.tile_pool(name="persist", bufs=1) as pp,
            tc.tile_pool(name="sb", bufs=2) as sb,
            tc.tile_pool(name="small", bufs=3) as sm,
        ):
            ident = pp.tile([P, P], F32)
            masks.make_identity(nc, ident[:])
            identb = pp.tile([P, P], BF16)
            nc.vector.tensor_copy(identb[:], ident[:])
            h2pre = pp.tile([P, NIB, DOUT], F32)

            # param shards -> full weights, gathered on-device.
            # (collectives cannot read IO tensors; stage via internal DRAM)
            nc.sync.dma_start(out=W0i[:, :],
                              in_=qblob[0, QW0_OFF:QW0_OFF + W0_SZ])
            nc.sync.dma_start(out=rp0wi[:, :],
                              in_=qblob[0, QRP0_OFF:QRP0_OFF + RP0_SZ])
            nc.sync.dma_start(out=W1i[:, :],
                              in_=qblob[0, QW1_OFF:QW1_OFF + W1_SZ])
            nc.sync.dma_start(out=rp1wi[:, :],
                              in_=qblob[0, QRP1_OFF:QRP1_OFF + RP1_SZ])
            nc.gpsimd.collective_compute(
                "AllGather", alu.bypass, replica_groups=groups,
                ins=[W0i[:, :].opt()], outs=[gW0[:, :].opt()])
            nc.gpsimd.collective_compute(
                "AllGather", alu.bypass, replica_groups=groups,
                ins=[rp0wi[:, :].opt()], outs=[grp0w[:, :].opt()])
            nc.gpsimd.collective_compute(
                "AllGather", alu.bypass, replica_groups=groups,
                ins=[W1i[:, :].opt()], outs=[gW1[:, :].opt()])
            nc.gpsimd.collective_compute(
                "AllGather", alu.bypass, replica_groups=groups,
                ins=[rp1wi[:, :].opt()], outs=[grp1w[:, :].opt()])

            def bcast(pool, dram_row, width, name):
                row = pool.tile([1, width], F32, tag="bc_row", bufs=1,
                                name=f"r_{name}")
                nc.sync.dma_start(out=row[:], in_=dram_row)
                out = pool.tile([P, width], F32, name=f"b_{name}")
                nc.gpsimd.partition_broadcast(out[:], row[0:1, :])
                return out

            def ln_elu(pool, x_ap, gb, bb, width, out_ap, do_elu):
                """LN over free dim; x_ap is clobbered as scratch (B0)."""
                b1 = pool.tile([P, width], F32, tag="ln_b1", bufs=1,
                               name="ln_b1")
                b2 = pool.tile([P, width], F32, tag="ln_b2", bufs=1,
                               name="ln_b2")
                s1 = sm.tile([P, 1], F32, tag="ln_s1", name="ln_s1")
                nc.vector.tensor_reduce(s1[:], x_ap, mybir.AxisListType.X,
                                        alu.add)
                negmean = sm.tile([P, 1], F32, tag="ln_nm", name="ln_nm")
                nc.vector.tensor_single_scalar(negmean[:], s1[:],
                                               -1.0 / width, alu.mult)
                nc.scalar.activation(b1[:], x_ap, AF.Identity,
                                     bias=negmean[:, 0:1])          # t
                ssq = sm.tile([P, 1], F32, tag="ln_ssq", name="ln_ssq")
                nc.scalar.activation(b2[:], b1[:], AF.Square,
                                     accum_out=ssq[:, 0:1])
                var = sm.tile([P, 1], F32, tag="ln_var", name="ln_var")
                nc.vector.tensor_scalar(var[:], ssq[:], 1.0 / width, EPS,
                                        alu.mult, alu.add)
                std = sm.tile([P, 1], F32, tag="ln_std", name="ln_std")
                nc.scalar.activation(std[:], var[:], AF.Sqrt)
                rstd = sm.tile([P, 1], F32, tag="ln_rstd", name="ln_rstd")
                nc.vector.reciprocal(rstd[:], std[:])
                nc.scalar.mul(b2[:], b1[:], rstd[:, 0:1])           # u
                nc.vector.tensor_tensor(b1[:], b2[:], gb, alu.mult)  # v
                if not do_elu:
                    nc.vector.tensor_tensor(out_ap, b1[:], bb, alu.add)
                    return
                nc.vector.tensor_tensor(b2[:], b1[:], bb, alu.add)   # w
                nc.vector.tensor_single_scalar(b1[:], b2[:], 0.0, alu.min)
                nc.scalar.activation(x_ap, b1[:], AF.Exp)            # -> B0
                nc.vector.tensor_single_scalar(b1[:], b2[:], 0.0, alu.max)
                nc.vector.scalar_tensor_tensor(out_ap, x_ap, -1.0, b1[:],
                                               alu.add, alu.add)

            def attention(lid, O, N1, g_out, gs_out, gs_in, dest, mean_heads):
                CX = O + 2
                with (
                    tc.tile_pool(name=f"att{lid}", bufs=1) as ap_,
                    tc.tile_pool(name=f"att{lid}_d", bufs=3) as ad,
                    tc.tile_pool(name=f"att{lid}_ps", bufs=1,
                                 space="PSUM") as aps,
                ):
                    ssb = []
                    for h in range(H):
                        row = sm.tile([1, NSH], F32, tag="ssrow",
                                      name=f"ssrow{lid}_{h}")
                        nc.sync.dma_start(
                            out=row[:],
                            in_=gs_in[h, :, 0:1].rearrange("q c -> c q"))
                        sbh = ap_.tile([P, NSH], F32, name=f"ssb{lid}_{h}")
                        nc.gpsimd.partition_broadcast(sbh[:], row[0:1, :])
                        ssb.append(sbh)
                    acc = [ap_.tile([P, NIB, O + 1], F32,
                                    name=f"acc{lid}_{hh}") for hh in range(H)]
                    whs = ap_.tile([P, CH, H, CX], BF16)
                    svs = ap_.tile([P, CH, H, 2], F32)
                    ewsTv = ewsTd.rearrange("(c p) q -> p c q", p=P)
                    for jc in range(NCHUNK):
                        ewsc = ad.tile([P, CH, NSH], BF16, tag="ewsc",
                                       bufs=1, name="ewsc")
                        nc.sync.dma_start(
                            out=ewsc[:],
                            in_=ewsTv[:, jc * CH:(jc + 1) * CH, :])
                        ewpc = ad.tile([P, CH, NSH], BF16, tag="ewpc",
                                       bufs=1, name="ewpc")
                        nc.vector.tensor_scalar(
                            ewpc[:], ewsc[:], 0.0, 1.0 / QSCALE,
                            alu.max, alu.mult)
                        mtc = ad.tile([P, CH, NSH], BF16, tag="mtc",
                                      bufs=1, name="mtc")
                        nc.vector.tensor_single_scalar(
                            mtc[:], ewsc[:], 0.0, alu.is_ge)
                        for jt in range(CH):
                            jg = jc * CH + jt
                            s, r = jg // NIB, jg % NIB
                            nc.sync.dma_start(
                                out=whs[:, jt, :, :],
                                in_=g_out[s, :, r * P:(r + 1) * P, :]
                                .rearrange("h p c -> p h c"))
                            nc.sync.dma_start(
                                out=svs[:, jt, :, :],
                                in_=gs_out[s, :, r * P:(r + 1) * P, :]
                                .rearrange("h p c -> p h c"))
                        for h in range(H):
                            psa = [aps.tile([P, N1], F32, tag=f"psa{qb}",
                                            name=f"psa_{qb}")
                                   for qb in range(NIB)]
                            psb = [aps.tile([P, 257], F32, tag=f"psb{qb}",
                                            name=f"psb_{qb}")
                                   for qb in range(NIB)]
                            for jt in range(CH):
                                e = ad.tile([P, NSH], F32, tag="e", name="e")
                                nc.scalar.activation(
                                    e[:], ssb[h][:, :], AF.Lrelu,
                                    bias=svs[:, jt, h, 1:2], alpha=ALPHA)
                                att = ad.tile([P, NSH], F32, tag="att",
                                              name="att")
                                nc.vector.tensor_tensor(
                                    att[:], e[:], ewpc[:, jt, :], alu.mult)
                                pt = ad.tile([P, NSH], BF16, tag="pt",
                                             name="pt")
                                nc.scalar.activation(pt[:], att[:], AF.Exp)
                                ptm = ad.tile([P, NSH], BF16, tag="ptm",
                                              name="ptm")
                                nc.vector.tensor_tensor(
                                    ptm[:], pt[:], mtc[:, jt, :], alu.mult)
                                for qb in range(NIB):
                                    lhs = ptm[:, qb * P:(qb + 1) * P]
                                    nc.tensor.matmul(
                                        psa[qb][:], lhs, whs[:, jt, h, 0:N1],
                                        start=(jt == 0), stop=(jt == CH - 1))
                                    nc.tensor.matmul(
                                        psb[qb][:], lhs,
                                        whs[:, jt, h, N1:N1 + 257],
                                        start=(jt == 0), stop=(jt == CH - 1))
                            for qb in range(NIB):
                                if jc == 0:
                                    nc.vector.tensor_copy(
                                        acc[h][:, qb, 0:N1], psa[qb][:])
                                    nc.vector.tensor_copy(
                                        acc[h][:, qb, N1:O + 1], psb[qb][:])
                                else:
                                    nc.vector.scalar_tensor_tensor(
                                        acc[h][:, qb, 0:N1], psa[qb][:], 0.0,
                                        acc[h][:, qb, 0:N1], alu.add, alu.add)
                                    nc.vector.scalar_tensor_tensor(
                                        acc[h][:, qb, N1:O + 1], psb[qb][:],
                                        0.0, acc[h][:, qb, N1:O + 1],
                                        alu.add, alu.add)
                    for h in range(H):
                        for qb in range(NIB):
                            den = sm.tile([P, 1], F32, tag="den", name="den")
                            if mean_heads:
                                nc.vector.tensor_single_scalar(
                                    den[:], acc[h][:, qb, O:O + 1], float(H),
                                    alu.mult)
                            else:
                                nc.vector.tensor_copy(
                                    den[:], acc[h][:, qb, O:O + 1])
                            rcp = sm.tile([P, 1], F32, tag="rcp", name="rcp")
                            nc.vector.reciprocal(rcp[:], den[:])
                            out_ap = (dest[:, qb, 0:O] if mean_heads else
                                      dest[:, qb, h * O:(h + 1) * O])
                            nc.vector.scalar_tensor_tensor(
                                out_ap, acc[h][:, qb, 0:O], rcp[:, 0:1],
                                out_ap, alu.mult, alu.add)

            # ---- poolX: h1pre / h1 / h1T ----
            with tc.tile_pool(name="poolX", bufs=1) as px:
                # ===== prep: transpose ews on device -> ewsTd (DRAM) =====
                with (
                    tc.tile_pool(name="prep", bufs=2) as pr,
                    tc.tile_pool(name="prep_ps", bufs=2, space="PSUM") as prps,
                ):
                    ewsTw = ewsTd.rearrange("(jt p) q -> p jt q", p=P)
                    for qb in range(NIB):
                        ewq = pr.tile([P, N], BF16, tag="ewq", name="ewq")
                        nc.gpsimd.dma_start(
                            out=ewq[:],
                            in_=qblob[0, QEWS_OFF + qb * P * N:
                                      QEWS_OFF + (qb + 1) * P * N]
                            .rearrange("(p j) -> p j", p=P))
                        st = pr.tile([P, NJT, P], BF16, tag="ewst",
                                     name="ewst")
                        for jt in range(NJT):
                            pstb = prps.tile([P, P], BF16, tag="prtb",
                                             name="prtb")
                            nc.tensor.transpose(
                                pstb[:], ewq[:, jt * P:(jt + 1) * P],
                                identb[:])
                            nc.scalar.copy(st[:, jt, :], pstb[:])
                        nc.sync.dma_start(
                            out=ewsTw[:, :, qb * P:(qb + 1) * P],
                            in_=st[:])

                h1pre = px.tile([P, NIB, F1], F32)

                # ===== Phase A =====
                with (
                    tc.tile_pool(name="phA", bufs=1) as pa,
                    tc.tile_pool(name="phA_ps", bufs=2, space="PSUM") as paps,
                ):
                    sc0 = pa.tile([P, SC_W0 // P], F32, name="sc0")
                    nc.gpsimd.dma_start(
                        out=sc0[:],
                        in_=blob[0, SW0_OFF:SW0_OFF + SC_W0]
                        .rearrange("(t p) -> p t", p=P))
                    src0 = pa.tile([P, SC_RP0 // P], F32, name="src0")
                    nc.gpsimd.dma_start(
                        out=src0[:],
                        in_=blob[0, SRP0_OFF:SRP0_OFF + SC_RP0]
                        .rearrange("(t p) -> p t", p=P))
                    a0b = bcast(pa, sblob[:, SB_A0:SB_A0 + H * 2 * HID],
                                H * 2 * HID, "a0")
                    a0b = a0b.rearrange("p (h c) -> p h c", h=H)
                    rp0bb = bcast(pa, sblob[:, SB_RP0B:SB_RP0B + F1],
                                  F1, "rp0b")
                    s_sb0 = pa.tile([P, H, NIB, 2], F32)

                    # transpose nf on device -> nfTbf
                    nfTbf = pa.tile([P, DIN // P, NSH], BF16)
                    with (
                        tc.tile_pool(name="nfp", bufs=1) as npr,
                        tc.tile_pool(name="nfp_ps", bufs=2,
                                     space="PSUM") as nps,
                    ):
                        nfsb = npr.tile([P, NIB, DIN], BF16)
                        nc.sync.dma_start(
                            out=nfsb[:],
                            in_=blob[0, NF_OFF:NF_OFF + NF_SZ]
                            .rearrange("(b p f) -> p b f", p=P, f=DIN))
                        for qb in range(NIB):
                            for kb in range(DIN // P):
                                pst = nps.tile([P, P], BF16, tag="nft",
                                               name="nft")
                                nc.tensor.transpose(
                                    pst[:],
                                    nfsb[:, qb, kb * P:(kb + 1) * P],
                                    identb[:])
                                nc.scalar.copy(
                                    nfTbf[:, kb, qb * P:(qb + 1) * P],
                                    pst[:])

                    for h in range(H):
                        psv = [paps.tile([P, HID], F32, tag=f"wh0ps{ib}",
                                         bufs=1, name=f"wh0ps_{ib}")
                               for ib in range(NIB)]
                        for k in range(DIN // P):
                            w0r = sb.tile([P, HID], BF16, tag="w0r",
                                          bufs=3, name="w0r")
                            nc.gpsimd.dma_start(
                                out=w0r[:],
                                in_=gW0[h * DIN + k * P:h * DIN + (k + 1) * P,
                                        :])
                            w0t = sb.tile([P, HID], BF16, tag="w0t",
                                          bufs=3, name="w0t")
                            t0i = h * (DIN // P) + k
                            nc.scalar.mul(w0t[:], w0r[:],
                                          sc0[:, t0i:t0i + 1])
                            for ib in range(NIB):
                                nc.tensor.matmul(
                                    psv[ib][:],
                                    nfTbf[:, k, ib * P:(ib + 1) * P],
                                    w0t[:],
                                    start=(k == 0), stop=(k == DIN // P - 1))
                        for ib in range(NIB):
                            ps = psv[ib]
                            whtmp = sb.tile([P, HID], F32, tag="whtmp",
                                            bufs=1, name="whtmp")
                            nc.scalar.copy(whtmp[:], ps[:])
                            for which in range(2):
                                tmp = sb.tile([P, HID], F32, tag="sred",
                                              bufs=1, name="sred")
                                nc.vector.tensor_tensor(
                                    tmp[:], whtmp[:],
                                    a0b[:, h, which * HID:(which + 1) * HID],
                                    alu.mult)
                                nc.vector.tensor_reduce(
                                    s_sb0[:, h, ib, which:which + 1], tmp[:],
                                    mybir.AxisListType.X, alu.add)
                            pack = sb.tile([P, C0], BF16, tag="pack0",
                                           name="pack")
                            nc.vector.tensor_copy(pack[:, 0:HID], whtmp[:])
                            nc.vector.memset(pack[:, HID:HID + 1], 1.0)
                            nc.vector.memset(pack[:, HID + 1:C0], 0.0)
                            nc.sync.dma_start(
                                out=g0_in[h, ib * P:(ib + 1) * P, :],
                                in_=pack[:])
                    nc.sync.dma_start(
                        out=g0s_in.rearrange("h (ib p) c -> p h ib c", p=P),
                        in_=s_sb0[:])
                    nc.gpsimd.collective_compute(
                        "AllGather", alu.bypass, replica_groups=groups,
                        ins=[g0_in[:, :, :].opt()],
                        outs=[g0_out[:, :, :, :].opt()])
                    nc.gpsimd.collective_compute(
                        "AllGather", alu.bypass, replica_groups=groups,
                        ins=[g0s_in[:, :, :].opt()],
                        outs=[g0s_out[:, :, :, :].opt()])

                    rp0wsb = pa.tile([P, DIN // P, F1], BF16)
                    nc.gpsimd.dma_start(
                        out=rp0wsb[:],
                        in_=grp0w.rearrange("(k p) o -> p k o", p=P))
                    for k in range(DIN // P):
                        nc.scalar.mul(rp0wsb[:, k, :], rp0wsb[:, k, :],
                                      src0[:, k:k + 1])
                    for ib in range(NIB):
                        for oc in range(4):
                            ps2 = paps.tile([P, 512], F32, tag="rp0ps",
                                            name="ps2")
                            for k in range(DIN // P):
                                nc.tensor.matmul(
                                    ps2[:], nfTbf[:, k, ib * P:(ib + 1) * P],
                                    rp0wsb[:, k, oc * 512:(oc + 1) * 512],
                                    start=(k == 0), stop=(k == DIN // P - 1))
                            nc.vector.tensor_tensor(
                                h1pre[:, ib, oc * 512:(oc + 1) * 512],
                                ps2[:], rp0bb[:, oc * 512:(oc + 1) * 512],
                                alu.add)

                attention(0, HID, 256, g0_out, g0s_out, g0s_in, h1pre, False)

                h1T = px.tile([P, F1 // P, NSH], BF16)
                # ===== LN0 + ELU -> h1, transpose -> h1T =====
                with tc.tile_pool(name="ln0p", bufs=1) as lp0:
                    ln0gb = bcast(lp0, sblob[:, SB_LN0G:SB_LN0G + F1],
                                  F1, "ln0g")
                    ln0bb = bcast(lp0, sblob[:, SB_LN0B:SB_LN0B + F1],
                                  F1, "ln0b")
                    for ib in range(NIB):
                        ln_elu(lp0, h1pre[:, ib, :], ln0gb[:, :],
                               ln0bb[:, :], F1, h1pre[:, ib, :], True)
                with tc.tile_pool(name="trps", bufs=2, space="PSUM") as tps:
                    for ib in range(NIB):
                        for fb in range(F1 // P):
                            pst = tps.tile([P, P], F32, tag="pst",
                                           name="pst")
                            nc.tensor.transpose(
                                pst[:], h1pre[:, ib, fb * P:(fb + 1) * P],
                                ident[:])
                            nc.scalar.copy(
                                h1T[:, fb, ib * P:(ib + 1) * P], pst[:])

                # ===== Phase B =====
                with (
                    tc.tile_pool(name="phB", bufs=1) as pb,
                    tc.tile_pool(name="phB_d", bufs=3) as pbd,
                    tc.tile_pool(name="phB_ps", bufs=1, space="PSUM") as pbps,
                ):
                    sc1 = pb.tile([P, SC_W1 // P], F32, name="sc1")
                    nc.gpsimd.dma_start(
                        out=sc1[:],
                        in_=blob[0, SW1_OFF:SW1_OFF + SC_W1]
                        .rearrange("(t p) -> p t", p=P))
                    scr1 = pb.tile([P, SC_RP1 // P], F32, name="scr1")
                    nc.gpsimd.dma_start(
                        out=scr1[:],
                        in_=blob[0, SRP1_OFF:SRP1_OFF + SC_RP1]
                        .rearrange("(t p) -> p t", p=P))
                    a1bs = [bcast(pb, sblob[:, SB_A1 + hh * 2 * DOUT:
                                            SB_A1 + (hh + 1) * 2 * DOUT],
                                  2 * DOUT, f"a1_{hh}") for hh in range(H)]
                    rp1bb = bcast(pb, sblob[:, SB_RP1B:SB_RP1B + DOUT],
                                  DOUT, "rp1b")
                    s_sb1 = pb.tile([P, H, NIB, 2], F32)
                    halves = ((0, 512), (512, DOUT))
                    for h in range(H):
                        psw = [pbps.tile([P, DOUT], F32, tag=f"wh1ps{ib}",
                                         name=f"wh1ps_{ib}")
                               for ib in range(NIB)]
                        for k in range(F1 // P):
                            w1r = pbd.tile([P, DOUT], BF16, tag="w1r",
                                           name="w1r")
                            nc.gpsimd.dma_start(
                                out=w1r[:],
                                in_=gW1[h * F1 + k * P:h * F1 + (k + 1) * P,
                                        :])
                            w1t = pbd.tile([P, DOUT], BF16, tag="w1t",
                                           name="w1t")
                            t1i = h * (F1 // P) + k
                            nc.scalar.mul(w1t[:], w1r[:],
                                          sc1[:, t1i:t1i + 1])
                            for ib in range(NIB):
                                for (o0, o1) in halves:
                                    nc.tensor.matmul(
                                        psw[ib][:, o0:o1],
                                        h1T[:, k, ib * P:(ib + 1) * P],
                                        w1t[:, o0:o1],
                                        start=(k == 0),
                                        stop=(k == F1 // P - 1))
                        for ib in range(NIB):
                            whtmp1 = sb.tile([P, DOUT], F32, tag="whtmp1",
                                             bufs=1, name="whtmp1")
                            nc.scalar.copy(whtmp1[:], psw[ib][:])
                            for which in range(2):
                                tmp = sb.tile([P, DOUT], F32, tag="sred1",
                                              bufs=1, name="tmp")
                                nc.vector.tensor_tensor(
                                    tmp[:], whtmp1[:],
                                    a1bs[h][:, which * DOUT:(which + 1) * DOUT],
                                    alu.mult)
                                nc.vector.tensor_reduce(
                                    s_sb1[:, h, ib, which:which + 1], tmp[:],
                                    mybir.AxisListType.X, alu.add)
                            pack1 = sb.tile([P, C1], BF16, tag="pack1",
                                            name="pack1")
                            nc.vector.tensor_copy(pack1[:, 0:DOUT],
                                                  whtmp1[:])
                            nc.vector.memset(pack1[:, DOUT:DOUT + 1], 1.0)
                            nc.vector.memset(pack1[:, DOUT + 1:C1], 0.0)
                            nc.sync.dma_start(
                                out=g1_in[h, ib * P:(ib + 1) * P, :],
                                in_=pack1[:])
                    nc.sync.dma_start(
                        out=g1s_in.rearrange("h (ib p) c -> p h ib c", p=P),
                        in_=s_sb1[:])
                    nc.gpsimd.collective_compute(
                        "AllGather", alu.bypass, replica_groups=groups,
                        ins=[g1_in[:, :, :].opt()],
                        outs=[g1_out[:, :, :, :].opt()])
                    nc.gpsimd.collective_compute(
                        "AllGather", alu.bypass, replica_groups=groups,
                        ins=[g1s_in[:, :, :].opt()],
                        outs=[g1s_out[:, :, :, :].opt()])

                    psr = [pbps.tile([P, DOUT], F32, tag=f"wh1ps{ib}",
                                     name=f"rp1ps_{ib}")
                           for ib in range(NIB)]
                    for k in range(F1 // P):
                        r1r = pbd.tile([P, DOUT], BF16, tag="r1r",
                                       name="r1r")
                        nc.gpsimd.dma_start(
                            out=r1r[:], in_=grp1w[k * P:(k + 1) * P, :])
                        r1t = pbd.tile([P, DOUT], BF16, tag="r1t",
                                       name="r1t")
                        nc.scalar.mul(r1t[:], r1r[:], scr1[:, k:k + 1])
                        for ib in range(NIB):
                            for (o0, o1) in halves:
                                nc.tensor.matmul(
                                    psr[ib][:, o0:o1],
                                    h1T[:, k, ib * P:(ib + 1) * P],
                                    r1t[:, o0:o1],
                                    start=(k == 0), stop=(k == F1 // P - 1))
                    for ib in range(NIB):
                        nc.vector.tensor_tensor(
                            h2pre[:, ib, :], psr[ib][:], rp1bb[:, :],
                            alu.add)

            attention(1, DOUT, 512, g1_out, g1s_out, g1s_in, h2pre, True)

            # ===== LN1 -> h2 out =====
            with tc.tile_pool(name="ln1p", bufs=1) as lp1:
                ln1gb = bcast(lp1, sblob[:, SB_LN1G:SB_LN1G + DOUT],
                              DOUT, "ln1g")
                ln1bb = bcast(lp1, sblob[:, SB_LN1B:SB_LN1B + DOUT],
                              DOUT, "ln1b")
                for ib in range(NIB):
                    o = sb.tile([P, DOUT], BF16, tag="hout", name="o")
                    ln_elu(lp1, h2pre[:, ib, :], ln1gb[:, :], ln1bb[:, :],
                           DOUT, o[:], False)
                    nc.sync.dma_start(out=h2[ib * P:(ib + 1) * P, :],
                                      in_=o[:])

    nc.finalize()
    return nc


_NC_CACHE = None


def _get_nc():
    global _NC_CACHE
    if _NC_CACHE is None:
        _NC_CACHE = build_nc()
    return _NC_CACHE


_SCRATCH = {}


def _scratch(name, shape, dtype):
    buf = _SCRATCH.get(name)
    if buf is None or buf.shape != shape or buf.dtype != dtype:
        buf = np.empty(shape, dtype)
        _SCRATCH[name] = buf
    return buf


def build_in_maps(node_features, adjacency, edge_weights, W0, a0, W1, a1,
                  rp0_w, rp0_b, rp1_w, rp1_b, ln0_g, ln0_b, ln1_g, ln1_b):
    bf = ml_dtypes.bfloat16
    nf = np.asarray(node_features, np.float32).astype(bf)
    adj = np.asarray(adjacency)
    ew = np.asarray(edge_weights, np.float32)

    # q[i,j] = round(ew*127) on edges (incl. diagonal); -1 off edges
    # (0 if ew == 0 exactly, preserving the reference's exp(0)=1 quirk).
    conn = adj != 0
    np.fill_diagonal(conn, True)
    fbuf = _scratch("fbuf", (N, N), np.float32)
    np.multiply(ew, np.float32(QSCALE), out=fbuf)
    np.add(fbuf, np.float32(0.5), out=fbuf)
    q = np.where(conn, fbuf.astype(np.int8), -(ew != 0).astype(np.int8))

    def qrows(x, rows, width):
        x = np.asarray(x, np.float32).reshape(rows, width)
        s = np.abs(x).max(axis=1, keepdims=True) * np.float32(1.0 / QSCALE)
        s[s == 0] = 1.0
        qw = np.rint(x / s).astype(np.int8)
        return qw, s.reshape(-1).astype(bf)

    w0q, w0s = qrows(W0, H * DIN, HID)
    w1q, w1s = qrows(W1, H * F1, DOUT)
    rp0q, rp0s = qrows(rp0_w, DIN, F1)
    rp1q, rp1s = qrows(rp1_w, F1, DOUT)

    blob = _scratch("blob", (NCORES, BLOB_SZ), bf)
    qb8 = _scratch("qb8", (NCORES, QBLOB_SZ), np.int8)
    for c in range(NCORES):
        rows = slice(c * NSH, (c + 1) * NSH)
        blob[c, NF_OFF:NF_OFF + NF_SZ] = nf[rows].reshape(-1)
        blob[c, SW0_OFF:SW0_OFF + SC_W0] = w0s
        blob[c, SW1_OFF:SW1_OFF + SC_W1] = w1s
        blob[c, SRP0_OFF:SRP0_OFF + SC_RP0] = rp0s
        blob[c, SRP1_OFF:SRP1_OFF + SC_RP1] = rp1s
        qb8[c, QEWS_OFF:QEWS_OFF + EWS_SZ] = q[rows].reshape(-1)
        qb8[c, QW0_OFF:QW0_OFF + W0_SZ] = \
            w0q[c * W0R:(c + 1) * W0R].reshape(-1)
        qb8[c, QW1_OFF:QW1_OFF + W1_SZ] = \
            w1q[c * W1R:(c + 1) * W1R].reshape(-1)
        qb8[c, QRP0_OFF:QRP0_OFF + RP0_SZ] = \
            rp0q[c * RP0R:(c + 1) * RP0R].reshape(-1)
        qb8[c, QRP1_OFF:QRP1_OFF + RP1_SZ] = \
            rp1q[c * RP1R:(c + 1) * RP1R].reshape(-1)

    sblob = np.concatenate([
        np.asarray(a0, np.float32).reshape(-1),
        np.asarray(a1, np.float32).reshape(-1),
        np.asarray(rp0_b, np.float32).reshape(-1),
        np.asarray(rp1_b, np.float32).reshape(-1),
        np.asarray(ln0_g, np.float32).reshape(-1),
        np.asarray(ln0_b, np.float32).reshape(-1),
        np.asarray(ln1_g, np.float32).reshape(-1),
        np.asarray(ln1_b, np.float32).reshape(-1),
    ]).reshape(1, SBLOB_SZ)

    return [{"blob": blob[c:c + 1], "qblob": qb8[c:c + 1], "sblob": sblob}
            for c in range(NCORES)]


def kernel(**inputs):
    in_maps = build_in_maps(**inputs)
    nc = _get_nc()
    res = run_bass_kernel_spmd(nc, in_maps, list(range(NCORES)))
    return np.concatenate([res.results[c]["h2"] for c in range(NCORES)],
                          axis=0).astype(np.float32)


# revision 25
# speedup vs baseline: 1.3616x; 1.3263x over previous
"""GAT-style 2-layer knowledge-graph encoder on 8 trn2 NeuronCores.

Sharding: query rows, 512 per core. Scores are built transposed ([j, q]) so
the exp'd attention matrix is directly the matmul lhsT (no PE transposes).
The softmax denominator comes from a ones-column appended to the gathered
Wh payload (an extra matmul output column, no reduction pass). Wh for each
layer is computed on the owning shard and AllGathered on-device (bf16).

Host->device traffic is minimized (the wall clock is transfer-bound over
the axon tunnel, ~80 MB/s):
  - weights ship as 1/8 row-shards in one bf16 blob and are AllGathered
    on device (collectives can't read IO tensors, so shards are staged
    through internal DRAM first);
  - edge weights + mask are fused into ONE int8 array in natural row
    layout:  q[i,j] = round(127*ew) on edges (incl. self-loops), else
    -1 (or 0 when ew == 0 exactly).  A gpsimd casting DMA widens int8
    -> bf16; after a PE transpose the device derives
      ewp = max(q, 0)/127   (score multiplier) and
      mt  = (q >= 0)        (post-exp multiplier),
    which reproduces the reference's exp(0)=1 behaviour when ew == 0
    (uniform quantization tracks exp's absolute-argument sensitivity
    better than bf16's relative error);
  - node features ship as bf16 rows and are PE-transposed on device;
  - h2 returns as bf16 and is widened to f32 on the host.
"""

import numpy as np
import ml_dtypes

try:
    import jax
    jax.config.update("jax_compilation_cache_dir", "/tmp/jax_ccache")
    jax.config.update("jax_persistent_cache_min_compile_time_secs", 0)
    jax.config.update("jax_persistent_cache_min_entry_size_bytes", 0)
except Exception:
    pass

import concourse.bass as bass
import concourse.bacc as bacc
import concourse.mybir as mybir
from concourse import tile, masks
from concourse.bass_utils import run_bass_kernel_spmd
from concourse.alu_op_type import AluOpType as alu

BF16 = mybir.dt.bfloat16
F32 = mybir.dt.float32
I8 = mybir.dt.int8

P = 128
NCORES = 8
N = 4096
NSH = 512          # rows per core
H = 4
DIN = 768
HID = 512
F1 = 2048
DOUT = 768
C0 = 514           # 512 Wh + ones + pad  (bf16)
C1 = 770           # 768 Wh + ones + pad  (bf16)
ALPHA = 0.2
EPS = 1e-5
NIB = NSH // P     # 4 row-blocks per core
CH = 4             # j-tiles per chunk
NCHUNK = (N // P) // CH
NJT = N // P       # 32 j-tiles
AF = mybir.ActivationFunctionType

# parameter shard row counts (full rows / 8)
W0R = H * DIN // NCORES      # 384 rows of [., HID]
W1R = H * F1 // NCORES       # 1024 rows of [., DOUT]
RP0R = DIN // NCORES         # 96 rows of [., F1]
RP1R = F1 // NCORES          # 256 rows of [., DOUT]

# bf16 blob layout (element offsets): one transfer per core
NF_SZ = NSH * DIN
EWS_SZ = NSH * N
W0_SZ = W0R * HID
W1_SZ = W1R * DOUT
RP0_SZ = RP0R * F1
RP1_SZ = RP1R * DOUT
NF_OFF = 0
W0_OFF = NF_OFF + NF_SZ
W1_OFF = W0_OFF + W0_SZ
RP0_OFF = W1_OFF + W1_SZ
RP1_OFF = RP0_OFF + RP0_SZ
BLOB_SZ = RP1_OFF + RP1_SZ
QSCALE = 127.0

# f32 smalls blob layout
SB_A0 = 0
SB_A1 = SB_A0 + H * 2 * HID
SB_RP0B = SB_A1 + H * 2 * DOUT
SB_RP1B = SB_RP0B + F1
SB_LN0G = SB_RP1B + DOUT
SB_LN0B = SB_LN0G + F1
SB_LN1G = SB_LN0B + F1
SB_LN1B = SB_LN1G + DOUT
SBLOB_SZ = SB_LN1B + DOUT


def build_nc():
    nc = bacc.Bacc(num_devices=NCORES)

    blob = nc.declare_dram_parameter("blob", [1, BLOB_SZ], BF16,
                                     isOutput=False)
    qblob = nc.declare_dram_parameter("qblob", [1, EWS_SZ], I8,
                                      isOutput=False)
    sblob = nc.declare_dram_parameter("sblob", [1, SBLOB_SZ], F32,
                                      isOutput=False)
    h2 = nc.declare_dram_parameter("h2", [NSH, DOUT], BF16, isOutput=True)

    ewsTd = nc.dram_tensor("ewsTd", [N, NSH], BF16)
    W0i = nc.dram_tensor("W0i", [W0R, HID], BF16)
    W1i = nc.dram_tensor("W1i", [W1R, DOUT], BF16)
    rp0wi = nc.dram_tensor("rp0wi", [RP0R, F1], BF16)
    rp1wi = nc.dram_tensor("rp1wi", [RP1R, DOUT], BF16)
    gW0 = nc.dram_tensor("gW0", [H * DIN, HID], BF16)
    gW1 = nc.dram_tensor("gW1", [H * F1, DOUT], BF16)
    grp0w = nc.dram_tensor("grp0w", [DIN, F1], BF16)
    grp1w = nc.dram_tensor("grp1w", [F1, DOUT], BF16)

    g0_in = nc.dram_tensor("g0_in", [H, NSH, C0], BF16)
    g0_out = nc.dram_tensor("g0_out", [NCORES, H, NSH, C0], BF16)
    g0s_in = nc.dram_tensor("g0s_in", [H, NSH, 2], F32)
    g0s_out = nc.dram_tensor("g0s_out", [NCORES, H, NSH, 2], F32)
    g1_in = nc.dram_tensor("g1_in", [H, NSH, C1], BF16)
    g1_out = nc.dram_tensor("g1_out", [NCORES, H, NSH, C1], BF16)
    g1s_in = nc.dram_tensor("g1s_in", [H, NSH, 2], F32)
    g1s_out = nc.dram_tensor("g1s_out", [NCORES, H, NSH, 2], F32)

    groups = [list(range(NCORES))]

    with tile.TileContext(nc) as tc:
        with (
            tc.tile_pool(name="persist", bufs=1) as pp,
            tc.tile_pool(name="sb", bufs=2) as sb,
            tc.tile_pool(name="small", bufs=3) as sm,
        ):
            ident = pp.tile([P, P], F32)
            masks.make_identity(nc, ident[:])
            identb = pp.tile([P, P], BF16)
            nc.vector.tensor_copy(identb[:], ident[:])
            h2pre = pp.tile([P, NIB, DOUT], F32)

            # param shards -> full weights, gathered on-device.
            # (collectives cannot read IO tensors; stage via internal DRAM)
            nc.sync.dma_start(out=W0i[:, :],
                              in_=blob[0, W0_OFF:W0_OFF + W0_SZ])
            nc.sync.dma_start(out=rp0wi[:, :],
                              in_=blob[0, RP0_OFF:RP0_OFF + RP0_SZ])
            nc.sync.dma_start(out=W1i[:, :],
                              in_=blob[0, W1_OFF:W1_OFF + W1_SZ])
            nc.sync.dma_start(out=rp1wi[:, :],
                              in_=blob[0, RP1_OFF:RP1_OFF + RP1_SZ])
            nc.gpsimd.collective_compute(
                "AllGather", alu.bypass, replica_groups=groups,
                ins=[W0i[:, :].opt()], outs=[gW0[:, :].opt()])
            nc.gpsimd.collective_compute(
                "AllGather", alu.bypass, replica_groups=groups,
                ins=[rp0wi[:, :].opt()], outs=[grp0w[:, :].opt()])
            nc.gpsimd.collective_compute(
                "AllGather", alu.bypass, replica_groups=groups,
                ins=[W1i[:, :].opt()], outs=[gW1[:, :].opt()])
            nc.gpsimd.collective_compute(
                "AllGather", alu.bypass, replica_groups=groups,
                ins=[rp1wi[:, :].opt()], outs=[grp1w[:, :].opt()])

            def bcast(pool, dram_row, width, name):
                row = pool.tile([1, width], F32, tag="bc_row", bufs=1,
                                name=f"r_{name}")
                nc.sync.dma_start(out=row[:], in_=dram_row)
                out = pool.tile([P, width], F32, name=f"b_{name}")
                nc.gpsimd.partition_broadcast(out[:], row[0:1, :])
                return out

            def ln_elu(pool, x_ap, gb, bb, width, out_ap, do_elu):
                """LN over free dim; x_ap is clobbered as scratch (B0)."""
                b1 = pool.tile([P, width], F32, tag="ln_b1", bufs=1,
                               name="ln_b1")
                b2 = pool.tile([P, width], F32, tag="ln_b2", bufs=1,
                               name="ln_b2")
                s1 = sm.tile([P, 1], F32, tag="ln_s1", name="ln_s1")
                nc.vector.tensor_reduce(s1[:], x_ap, mybir.AxisListType.X,
                                        alu.add)
                negmean = sm.tile([P, 1], F32, tag="ln_nm", name="ln_nm")
                nc.vector.tensor_single_scalar(negmean[:], s1[:],
                                               -1.0 / width, alu.mult)
                nc.scalar.activation(b1[:], x_ap, AF.Identity,
                                     bias=negmean[:, 0:1])          # t
                ssq = sm.tile([P, 1], F32, tag="ln_ssq", name="ln_ssq")
                nc.scalar.activation(b2[:], b1[:], AF.Square,
                                     accum_out=ssq[:, 0:1])
                var = sm.tile([P, 1], F32, tag="ln_var", name="ln_var")
                nc.vector.tensor_scalar(var[:], ssq[:], 1.0 / width, EPS,
                                        alu.mult, alu.add)
                std = sm.tile([P, 1], F32, tag="ln_std", name="ln_std")
                nc.scalar.activation(std[:], var[:], AF.Sqrt)
                rstd = sm.tile([P, 1], F32, tag="ln_rstd", name="ln_rstd")
                nc.vector.reciprocal(rstd[:], std[:])
                nc.scalar.mul(b2[:], b1[:], rstd[:, 0:1])           # u
                nc.vector.tensor_tensor(b1[:], b2[:], gb, alu.mult)  # v
                if not do_elu:
                    nc.vector.tensor_tensor(out_ap, b1[:], bb, alu.add)
                    return
                nc.vector.tensor_tensor(b2[:], b1[:], bb, alu.add)   # w
                nc.vector.tensor_single_scalar(b1[:], b2[:], 0.0, alu.min)
                nc.scalar.activation(x_ap, b1[:], AF.Exp)            # -> B0
                nc.vector.tensor_single_scalar(b1[:], b2[:], 0.0, alu.max)
                nc.vector.scalar_tensor_tensor(out_ap, x_ap, -1.0, b1[:],
                                               alu.add, alu.add)

            def attention(lid, O, N1, g_out, gs_out, gs_in, dest, mean_heads):
                CX = O + 2
                with (
                    tc.tile_pool(name=f"att{lid}", bufs=1) as ap_,
                    tc.tile_pool(name=f"att{lid}_d", bufs=3) as ad,
                    tc.tile_pool(name=f"att{lid}_ps", bufs=1,
                                 space="PSUM") as aps,
                ):
                    ssb = []
                    for h in range(H):
                        row = sm.tile([1, NSH], F32, tag="ssrow",
                                      name=f"ssrow{lid}_{h}")
                        nc.sync.dma_start(
                            out=row[:],
                            in_=gs_in[h, :, 0:1].rearrange("q c -> c q"))
                        sbh = ap_.tile([P, NSH], F32, name=f"ssb{lid}_{h}")
                        nc.gpsimd.partition_broadcast(sbh[:], row[0:1, :])
                        ssb.append(sbh)
                    acc = [ap_.tile([P, NIB, O + 1], F32,
                                    name=f"acc{lid}_{hh}") for hh in range(H)]
                    whs = ap_.tile([P, CH, H, CX], BF16)
                    svs = ap_.tile([P, CH, H, 2], F32)
                    ewsTv = ewsTd.rearrange("(c p) q -> p c q", p=P)
                    for jc in range(NCHUNK):
                        ewsc = ad.tile([P, CH, NSH], BF16, tag="ewsc",
                                       bufs=1, name="ewsc")
                        nc.sync.dma_start(
                            out=ewsc[:],
                            in_=ewsTv[:, jc * CH:(jc + 1) * CH, :])
                        ewpc = ad.tile([P, CH, NSH], BF16, tag="ewpc",
                                       bufs=1, name="ewpc")
                        nc.vector.tensor_scalar(
                            ewpc[:], ewsc[:], 0.0, 1.0 / QSCALE,
                            alu.max, alu.mult)
                        mtc = ad.tile([P, CH, NSH], BF16, tag="mtc",
                                      bufs=1, name="mtc")
                        nc.vector.tensor_single_scalar(
                            mtc[:], ewsc[:], 0.0, alu.is_ge)
                        for jt in range(CH):
                            jg = jc * CH + jt
                            s, r = jg // NIB, jg % NIB
                            nc.sync.dma_start(
                                out=whs[:, jt, :, :],
                                in_=g_out[s, :, r * P:(r + 1) * P, :]
                                .rearrange("h p c -> p h c"))
                            nc.sync.dma_start(
                                out=svs[:, jt, :, :],
                                in_=gs_out[s, :, r * P:(r + 1) * P, :]
                                .rearrange("h p c -> p h c"))
                        for h in range(H):
                            psa = [aps.tile([P, N1], F32, tag=f"psa{qb}",
                                            name=f"psa_{qb}")
                                   for qb in range(NIB)]
                            psb = [aps.tile([P, 257], F32, tag=f"psb{qb}",
                                            name=f"psb_{qb}")
                                   for qb in range(NIB)]
                            for jt in range(CH):
                                e = ad.tile([P, NSH], F32, tag="e", name="e")
                                nc.scalar.activation(
                                    e[:], ssb[h][:, :], AF.Lrelu,
                                    bias=svs[:, jt, h, 1:2], alpha=ALPHA)
                                att = ad.tile([P, NSH], F32, tag="att",
                                              name="att")
                                nc.vector.tensor_tensor(
                                    att[:], e[:], ewpc[:, jt, :], alu.mult)
                                pt = ad.tile([P, NSH], BF16, tag="pt",
                                             name="pt")
                                nc.scalar.activation(pt[:], att[:], AF.Exp)
                                ptm = ad.tile([P, NSH], BF16, tag="ptm",
                                              name="ptm")
                                nc.vector.tensor_tensor(
                                    ptm[:], pt[:], mtc[:, jt, :], alu.mult)
                                for qb in range(NIB):
                                    lhs = ptm[:, qb * P:(qb + 1) * P]
                                    nc.tensor.matmul(
                                        psa[qb][:], lhs, whs[:, jt, h, 0:N1],
                                        start=(jt == 0), stop=(jt == CH - 1))
                                    nc.tensor.matmul(
                                        psb[qb][:], lhs,
                                        whs[:, jt, h, N1:N1 + 257],
                                        start=(jt == 0), stop=(jt == CH - 1))
                            for qb in range(NIB):
                                if jc == 0:
                                    nc.vector.tensor_copy(
                                        acc[h][:, qb, 0:N1], psa[qb][:])
                                    nc.vector.tensor_copy(
                                        acc[h][:, qb, N1:O + 1], psb[qb][:])
                                else:
                                    nc.vector.scalar_tensor_tensor(
                                        acc[h][:, qb, 0:N1], psa[qb][:], 0.0,
                                        acc[h][:, qb, 0:N1], alu.add, alu.add)
                                    nc.vector.scalar_tensor_tensor(
                                        acc[h][:, qb, N1:O + 1], psb[qb][:],
                                        0.0, acc[h][:, qb, N1:O + 1],
                                        alu.add, alu.add)
                    for h in range(H):
                        for qb in range(NIB):
                            den = sm.tile([P, 1], F32, tag="den", name="den")
                            if mean_heads:
                                nc.vector.tensor_single_scalar(
                                    den[:], acc[h][:, qb, O:O + 1], float(H),
                                    alu.mult)
                            else:
                                nc.vector.tensor_copy(
                                    den[:], acc[h][:, qb, O:O + 1])
                            rcp = sm.tile([P, 1], F32, tag="rcp", name="rcp")
                            nc.vector.reciprocal(rcp[:], den[:])
                            out_ap = (dest[:, qb, 0:O] if mean_heads else
                                      dest[:, qb, h * O:(h + 1) * O])
                            nc.vector.scalar_tensor_tensor(
                                out_ap, acc[h][:, qb, 0:O], rcp[:, 0:1],
                                out_ap, alu.mult, alu.add)

            # ---- poolX: h1pre / h1 / h1T ----
            with tc.tile_pool(name="poolX", bufs=1) as px:
                # ===== prep: transpose ews on device -> ewsTd (DRAM) =====
                with (
                    tc.tile_pool(name="prep", bufs=2) as pr,
                    tc.tile_pool(name="prep_ps", bufs=2, space="PSUM") as prps,
                ):
                    ewsTw = ewsTd.rearrange("(jt p) q -> p jt q", p=P)
                    for qb in range(NIB):
                        ewq = pr.tile([P, N], BF16, tag="ewq", name="ewq")
                        nc.gpsimd.dma_start(
                            out=ewq[:],
                            in_=qblob[0, qb * P * N:(qb + 1) * P * N]
                            .rearrange("(p j) -> p j", p=P))
                        st = pr.tile([P, NJT, P], BF16, tag="ewst",
                                     name="ewst")
                        for jt in range(NJT):
                            pstb = prps.tile([P, P], BF16, tag="prtb",
                                             name="prtb")
                            nc.tensor.transpose(
                                pstb[:], ewq[:, jt * P:(jt + 1) * P],
                                identb[:])
                            nc.scalar.copy(st[:, jt, :], pstb[:])
                        nc.sync.dma_start(
                            out=ewsTw[:, :, qb * P:(qb + 1) * P],
                            in_=st[:])

                h1pre = px.tile([P, NIB, F1], F32)

                # ===== Phase A =====
                with (
                    tc.tile_pool(name="phA", bufs=1) as pa,
                    tc.tile_pool(name="phA_ps", bufs=2, space="PSUM") as paps,
                ):
                    a0b = bcast(pa, sblob[:, SB_A0:SB_A0 + H * 2 * HID],
                                H * 2 * HID, "a0")
                    a0b = a0b.rearrange("p (h c) -> p h c", h=H)
                    rp0bb = bcast(pa, sblob[:, SB_RP0B:SB_RP0B + F1],
                                  F1, "rp0b")
                    s_sb0 = pa.tile([P, H, NIB, 2], F32)

                    # transpose nf on device -> nfTbf
                    nfTbf = pa.tile([P, DIN // P, NSH], BF16)
                    with (
                        tc.tile_pool(name="nfp", bufs=1) as npr,
                        tc.tile_pool(name="nfp_ps", bufs=2,
                                     space="PSUM") as nps,
                    ):
                        nfsb = npr.tile([P, NIB, DIN], BF16)
                        nc.sync.dma_start(
                            out=nfsb[:],
                            in_=blob[0, NF_OFF:NF_OFF + NF_SZ]
                            .rearrange("(b p f) -> p b f", p=P, f=DIN))
                        for qb in range(NIB):
                            for kb in range(DIN // P):
                                pst = nps.tile([P, P], BF16, tag="nft",
                                               name="nft")
                                nc.tensor.transpose(
                                    pst[:],
                                    nfsb[:, qb, kb * P:(kb + 1) * P],
                                    identb[:])
                                nc.scalar.copy(
                                    nfTbf[:, kb, qb * P:(qb + 1) * P],
                                    pst[:])

                    for h in range(H):
                        psv = [paps.tile([P, HID], F32, tag=f"wh0ps{ib}",
                                         bufs=1, name=f"wh0ps_{ib}")
                               for ib in range(NIB)]
                        for k in range(DIN // P):
                            w0t = sb.tile([P, HID], BF16, tag="w0t",
                                          bufs=3, name="w0t")
                            nc.sync.dma_start(
                                out=w0t[:],
                                in_=gW0[h * DIN + k * P:h * DIN + (k + 1) * P,
                                        :])
                            for ib in range(NIB):
                                nc.tensor.matmul(
                                    psv[ib][:],
                                    nfTbf[:, k, ib * P:(ib + 1) * P],
                                    w0t[:],
                                    start=(k == 0), stop=(k == DIN // P - 1))
                        for ib in range(NIB):
                            ps = psv[ib]
                            whtmp = sb.tile([P, HID], F32, tag="whtmp",
                                            bufs=1, name="whtmp")
                            nc.scalar.copy(whtmp[:], ps[:])
                            for which in range(2):
                                tmp = sb.tile([P, HID], F32, tag="sred",
                                              bufs=1, name="sred")
                                nc.vector.tensor_tensor(
                                    tmp[:], whtmp[:],
                                    a0b[:, h, which * HID:(which + 1) * HID],
                                    alu.mult)
                                nc.vector.tensor_reduce(
                                    s_sb0[:, h, ib, which:which + 1], tmp[:],
                                    mybir.AxisListType.X, alu.add)
                            pack = sb.tile([P, C0], BF16, tag="pack0",
                                           name="pack")
                            nc.vector.tensor_copy(pack[:, 0:HID], whtmp[:])
                            nc.vector.memset(pack[:, HID:HID + 1], 1.0)
                            nc.vector.memset(pack[:, HID + 1:C0], 0.0)
                            nc.sync.dma_start(
                                out=g0_in[h, ib * P:(ib + 1) * P, :],
                                in_=pack[:])
                    nc.sync.dma_start(
                        out=g0s_in.rearrange("h (ib p) c -> p h ib c", p=P),
                        in_=s_sb0[:])
                    nc.gpsimd.collective_compute(
                        "AllGather", alu.bypass, replica_groups=groups,
                        ins=[g0_in[:, :, :].opt()],
                        outs=[g0_out[:, :, :, :].opt()])
                    nc.gpsimd.collective_compute(
                        "AllGather", alu.bypass, replica_groups=groups,
                        ins=[g0s_in[:, :, :].opt()],
                        outs=[g0s_out[:, :, :, :].opt()])

                    rp0wsb = pa.tile([P, DIN // P, F1], BF16)
                    nc.sync.dma_start(
                        out=rp0wsb[:],
                        in_=grp0w.rearrange("(k p) o -> p k o", p=P))
                    for ib in range(NIB):
                        for oc in range(4):
                            ps2 = paps.tile([P, 512], F32, tag="rp0ps",
                                            name="ps2")
                            for k in range(DIN // P):
                                nc.tensor.matmul(
                                    ps2[:], nfTbf[:, k, ib * P:(ib + 1) * P],
                                    rp0wsb[:, k, oc * 512:(oc + 1) * 512],
                                    start=(k == 0), stop=(k == DIN // P - 1))
                            nc.vector.tensor_tensor(
                                h1pre[:, ib, oc * 512:(oc + 1) * 512],
                                ps2[:], rp0bb[:, oc * 512:(oc + 1) * 512],
                                alu.add)

                attention(0, HID, 256, g0_out, g0s_out, g0s_in, h1pre, False)

                h1T = px.tile([P, F1 // P, NSH], BF16)
                # ===== LN0 + ELU -> h1, transpose -> h1T =====
                with tc.tile_pool(name="ln0p", bufs=1) as lp0:
                    ln0gb = bcast(lp0, sblob[:, SB_LN0G:SB_LN0G + F1],
                                  F1, "ln0g")
                    ln0bb = bcast(lp0, sblob[:, SB_LN0B:SB_LN0B + F1],
                                  F1, "ln0b")
                    for ib in range(NIB):
                        ln_elu(lp0, h1pre[:, ib, :], ln0gb[:, :],
                               ln0bb[:, :], F1, h1pre[:, ib, :], True)
                with tc.tile_pool(name="trps", bufs=2, space="PSUM") as tps:
                    for ib in range(NIB):
                        for fb in range(F1 // P):
                            pst = tps.tile([P, P], F32, tag="pst",
                                           name="pst")
                            nc.tensor.transpose(
                                pst[:], h1pre[:, ib, fb * P:(fb + 1) * P],
                                ident[:])
                            nc.scalar.copy(
                                h1T[:, fb, ib * P:(ib + 1) * P], pst[:])

                # ===== Phase B =====
                with (
                    tc.tile_pool(name="phB", bufs=1) as pb,
                    tc.tile_pool(name="phB_d", bufs=3) as pbd,
                    tc.tile_pool(name="phB_ps", bufs=1, space="PSUM") as pbps,
                ):
                    a1bs = [bcast(pb, sblob[:, SB_A1 + hh * 2 * DOUT:
                                            SB_A1 + (hh + 1) * 2 * DOUT],
                                  2 * DOUT, f"a1_{hh}") for hh in range(H)]
                    rp1bb = bcast(pb, sblob[:, SB_RP1B:SB_RP1B + DOUT],
                                  DOUT, "rp1b")
                    s_sb1 = pb.tile([P, H, NIB, 2], F32)
                    halves = ((0, 512), (512, DOUT))
                    for h in range(H):
                        psw = [pbps.tile([P, DOUT], F32, tag=f"wh1ps{ib}",
                                         name=f"wh1ps_{ib}")
                               for ib in range(NIB)]
                        for k in range(F1 // P):
                            w1t = pbd.tile([P, DOUT], BF16, tag="w1t",
                                           name="w1t")
                            nc.sync.dma_start(
                                out=w1t[:],
                                in_=gW1[h * F1 + k * P:h * F1 + (k + 1) * P,
                                        :])
                            for ib in range(NIB):
                                for (o0, o1) in halves:
                                    nc.tensor.matmul(
                                        psw[ib][:, o0:o1],
                                        h1T[:, k, ib * P:(ib + 1) * P],
                                        w1t[:, o0:o1],
                                        start=(k == 0),
                                        stop=(k == F1 // P - 1))
                        for ib in range(NIB):
                            whtmp1 = sb.tile([P, DOUT], F32, tag="whtmp1",
                                             bufs=1, name="whtmp1")
                            nc.scalar.copy(whtmp1[:], psw[ib][:])
                            for which in range(2):
                                tmp = sb.tile([P, DOUT], F32, tag="sred1",
                                              bufs=1, name="tmp")
                                nc.vector.tensor_tensor(
                                    tmp[:], whtmp1[:],
                                    a1bs[h][:, which * DOUT:(which + 1) * DOUT],
                                    alu.mult)
                                nc.vector.tensor_reduce(
                                    s_sb1[:, h, ib, which:which + 1], tmp[:],
                                    mybir.AxisListType.X, alu.add)
                            pack1 = sb.tile([P, C1], BF16, tag="pack1",
                                            name="pack1")
                            nc.vector.tensor_copy(pack1[:, 0:DOUT],
                                                  whtmp1[:])
                            nc.vector.memset(pack1[:, DOUT:DOUT + 1], 1.0)
                            nc.vector.memset(pack1[:, DOUT + 1:C1], 0.0)
                            nc.sync.dma_start(
                                out=g1_in[h, ib * P:(ib + 1) * P, :],
                                in_=pack1[:])
                    nc.sync.dma_start(
                        out=g1s_in.rearrange("h (ib p) c -> p h ib c", p=P),
                        in_=s_sb1[:])
                    nc.gpsimd.collective_compute(
                        "AllGather", alu.bypass, replica_groups=groups,
                        ins=[g1_in[:, :, :].opt()],
                        outs=[g1_out[:, :, :, :].opt()])
                    nc.gpsimd.collective_compute(
                        "AllGather", alu.bypass, replica_groups=groups,
                        ins=[g1s_in[:, :, :].opt()],
                        outs=[g1s_out[:, :, :, :].opt()])

                    psr = [pbps.tile([P, DOUT], F32, tag=f"wh1ps{ib}",
                                     name=f"rp1ps_{ib}")
                           for ib in range(NIB)]
                    for k in range(F1 // P):
                        r1t = pbd.tile([P, DOUT], BF16, tag="r1t",
                                       name="r1t")
                        nc.sync.dma_start(
                            out=r1t[:], in_=grp1w[k * P:(k + 1) * P, :])
                        for ib in range(NIB):
                            for (o0, o1) in halves:
                                nc.tensor.matmul(
                                    psr[ib][:, o0:o1],
                                    h1T[:, k, ib * P:(ib + 1) * P],
                                    r1t[:, o0:o1],
                                    start=(k == 0), stop=(k == F1 // P - 1))
                    for ib in range(NIB):
                        nc.vector.tensor_tensor(
                            h2pre[:, ib, :], psr[ib][:], rp1bb[:, :],
                            alu.add)

            attention(1, DOUT, 512, g1_out, g1s_out, g1s_in, h2pre, True)

            # ===== LN1 -> h2 out =====
            with tc.tile_pool(name="ln1p", bufs=1) as lp1:
                ln1gb = bcast(lp1, sblob[:, SB_LN1G:SB_LN1G + DOUT],
                              DOUT, "ln1g")
                ln1bb = bcast(lp1, sblob[:, SB_LN1B:SB_LN1B + DOUT],
                              DOUT, "ln1b")
                for ib in range(NIB):
                    o = sb.tile([P, DOUT], BF16, tag="hout", name="o")
                    ln_elu(lp1, h2pre[:, ib, :], ln1gb[:, :], ln1bb[:, :],
                           DOUT, o[:], False)
                    nc.sync.dma_start(out=h2[ib * P:(ib + 1) * P, :],
                                      in_=o[:])

    nc.finalize()
    return nc


_NC_CACHE = None


def _get_nc():
    global _NC_CACHE
    if _NC_CACHE is None:
        _NC_CACHE = build_nc()
    return _NC_CACHE


_SCRATCH = {}


def _scratch(name, shape, dtype):
    buf = _SCRATCH.get(name)
    if buf is None or buf.shape != shape or buf.dtype != dtype:
        buf = np.empty(shape, dtype)
        _SCRATCH[name] = buf
    return buf


def build_in_maps(node_features, adjacency, edge_weights, W0, a0, W1, a1,
                  rp0_w, rp0_b, rp1_w, rp1_b, ln0_g, ln0_b, ln1_g, ln1_b):
    bf = ml_dtypes.bfloat16
    nf = np.asarray(node_features, np.float32).astype(bf)
    adj = np.asarray(adjacency)
    ew = np.asarray(edge_weights, np.float32)

    # q[i,j] = round(ew*127) on edges (incl. diagonal); -1 off edges
    # (0 if ew == 0 exactly, preserving the reference's exp(0)=1 quirk).
    conn = adj != 0
    np.fill_diagonal(conn, True)
    fbuf = _scratch("fbuf", (N, N), np.float32)
    np.multiply(ew, np.float32(QSCALE), out=fbuf)
    np.add(fbuf, np.float32(0.5), out=fbuf)
    q = np.where(conn, fbuf.astype(np.int8), -(ew != 0).astype(np.int8))

    w0 = np.asarray(W0, np.float32).reshape(H * DIN, HID).astype(bf)
    w1 = np.asarray(W1, np.float32).reshape(H * F1, DOUT).astype(bf)
    rp0w = np.asarray(rp0_w, np.float32).astype(bf)
    rp1w = np.asarray(rp1_w, np.float32).astype(bf)

    blob = _scratch("blob", (NCORES, BLOB_SZ), bf)
    for c in range(NCORES):
        rows = slice(c * NSH, (c + 1) * NSH)
        blob[c, NF_OFF:NF_OFF + NF_SZ] = nf[rows].reshape(-1)
        blob[c, W0_OFF:W0_OFF + W0_SZ] = \
            w0[c * W0R:(c + 1) * W0R].reshape(-1)
        blob[c, W1_OFF:W1_OFF + W1_SZ] = \
            w1[c * W1R:(c + 1) * W1R].reshape(-1)
        blob[c, RP0_OFF:RP0_OFF + RP0_SZ] = \
            rp0w[c * RP0R:(c + 1) * RP0R].reshape(-1)
        blob[c, RP1_OFF:RP1_OFF + RP1_SZ] = \
            rp1w[c * RP1R:(c + 1) * RP1R].reshape(-1)

    sblob = np.concatenate([
        np.asarray(a0, np.float32).reshape(-1),
        np.asarray(a1, np.float32).reshape(-1),
        np.asarray(rp0_b, np.float32).reshape(-1),
        np.asarray(rp1_b, np.float32).reshape(-1),
        np.asarray(ln0_g, np.float32).reshape(-1),
        np.asarray(ln0_b, np.float32).reshape(-1),
        np.asarray(ln1_g, np.float32).reshape(-1),
        np.asarray(ln1_b, np.float32).reshape(-1),
    ]).reshape(1, SBLOB_SZ)

    qb = q.reshape(NCORES, 1, EWS_SZ)
    return [{"blob": blob[c:c + 1], "qblob": qb[c], "sblob": sblob}
            for c in range(NCORES)]


def kernel(**inputs):
    in_maps = build_in_maps(**inputs)
    nc = _get_nc()
    res = run_bass_kernel_spmd(nc, in_maps, list(range(NCORES)))
    return np.concatenate([res.results[c]["h2"] for c in range(NCORES)],
                          axis=0).astype(np.float32)


# revision 26
# speedup vs baseline: 1.3639x; 1.0017x over previous
"""GAT-style 2-layer knowledge-graph encoder on 8 trn2 NeuronCores.

Sharding: query rows, 512 per core. Scores are built transposed ([j, q]) so
the exp'd attention matrix is directly the matmul lhsT (no PE transposes).
The softmax denominator comes from a ones-column appended to the gathered
Wh payload (an extra matmul output column, no reduction pass). Wh for each
layer is computed on the owning shard and AllGathered on-device (bf16).

Host->device traffic is minimized (the wall clock is transfer-bound over
the axon tunnel, ~80 MB/s):
  - weights ship as 1/8 row-shards in one bf16 blob and are AllGathered
    on device (collectives can't read IO tensors, so shards are staged
    through internal DRAM first);
  - edge weights + mask are fused into ONE int8 array in natural row
    layout:  q[i,j] = round(127*ew) on edges (incl. self-loops), else
    -1 (or 0 when ew == 0 exactly).  A gpsimd casting DMA widens int8
    -> bf16; after a PE transpose the device derives
      ewp = max(q, 0)/127   (score multiplier) and
      mt  = (q >= 0)        (post-exp multiplier),
    which reproduces the reference's exp(0)=1 behaviour when ew == 0
    (uniform quantization tracks exp's absolute-argument sensitivity
    better than bf16's relative error);
  - node features ship as bf16 rows and are PE-transposed on device;
  - h2 returns as bf16 and is widened to f32 on the host.
"""

import numpy as np
import ml_dtypes

try:
    import jax
    jax.config.update("jax_compilation_cache_dir", "/tmp/jax_ccache")
    jax.config.update("jax_persistent_cache_min_compile_time_secs", 0)
    jax.config.update("jax_persistent_cache_min_entry_size_bytes", 0)
except Exception:
    pass

import concourse.bass as bass
import concourse.bacc as bacc
import concourse.mybir as mybir
from concourse import tile, masks
from concourse.bass_utils import run_bass_kernel_spmd
from concourse.alu_op_type import AluOpType as alu

BF16 = mybir.dt.bfloat16
F32 = mybir.dt.float32
I8 = mybir.dt.int8

P = 128
NCORES = 8
N = 4096
NSH = 512          # rows per core
H = 4
DIN = 768
HID = 512
F1 = 2048
DOUT = 768
C0 = 514           # 512 Wh + ones + pad  (bf16)
C1 = 770           # 768 Wh + ones + pad  (bf16)
ALPHA = 0.2
EPS = 1e-5
NIB = NSH // P     # 4 row-blocks per core
CH = 4             # j-tiles per chunk
NCHUNK = (N // P) // CH
NJT = N // P       # 32 j-tiles
AF = mybir.ActivationFunctionType

# parameter shard row counts (full rows / 8)
W0R = H * DIN // NCORES      # 384 rows of [., HID]
W1R = H * F1 // NCORES       # 1024 rows of [., DOUT]
RP0R = DIN // NCORES         # 96 rows of [., F1]
RP1R = F1 // NCORES          # 256 rows of [., DOUT]

NF_SZ = NSH * DIN
EWS_SZ = NSH * N
W0_SZ = W0R * HID
W1_SZ = W1R * DOUT
RP0_SZ = RP0R * F1
RP1_SZ = RP1R * DOUT
QSCALE = 127.0

# int8 qblob layout (element offsets): ews rows + weight shards
QEWS_OFF = 0
QW0_OFF = QEWS_OFF + EWS_SZ
QW1_OFF = QW0_OFF + W0_SZ
QRP0_OFF = QW1_OFF + W1_SZ
QRP1_OFF = QRP0_OFF + RP0_SZ
QBLOB_SZ = QRP1_OFF + RP1_SZ

# bf16 blob layout: nf rows + FULL per-row dequant scales (replicated)
SC_W0 = H * DIN        # 3072 rows
SC_W1 = H * F1         # 8192 rows
SC_RP0 = DIN           # 768 rows
SC_RP1 = F1            # 2048 rows
NF_OFF = 0
SW0_OFF = NF_OFF + NF_SZ
SW1_OFF = SW0_OFF + SC_W0
SRP0_OFF = SW1_OFF + SC_W1
SRP1_OFF = SRP0_OFF + SC_RP0
BLOB_SZ = SRP1_OFF + SC_RP1

# f32 smalls blob layout
SB_A0 = 0
SB_A1 = SB_A0 + H * 2 * HID
SB_RP0B = SB_A1 + H * 2 * DOUT
SB_RP1B = SB_RP0B + F1
SB_LN0G = SB_RP1B + DOUT
SB_LN0B = SB_LN0G + F1
SB_LN1G = SB_LN0B + F1
SB_LN1B = SB_LN1G + DOUT
SBLOB_SZ = SB_LN1B + DOUT


def build_nc():
    nc = bacc.Bacc(num_devices=NCORES)

    blob = nc.declare_dram_parameter("blob", [1, BLOB_SZ], BF16,
                                     isOutput=False)
    qblob = nc.declare_dram_parameter("qblob", [1, QBLOB_SZ], I8,
                                      isOutput=False)
    sblob = nc.declare_dram_parameter("sblob", [1, SBLOB_SZ], F32,
                                      isOutput=False)
    h2 = nc.declare_dram_parameter("h2", [NSH, DOUT], BF16, isOutput=True)

    ewsTd = nc.dram_tensor("ewsTd", [N, NSH], BF16)
    W0i = nc.dram_tensor("W0i", [W0R, HID], I8)
    W1i = nc.dram_tensor("W1i", [W1R, DOUT], I8)
    rp0wi = nc.dram_tensor("rp0wi", [RP0R, F1], I8)
    rp1wi = nc.dram_tensor("rp1wi", [RP1R, DOUT], I8)
    gW0 = nc.dram_tensor("gW0", [H * DIN, HID], I8)
    gW1 = nc.dram_tensor("gW1", [H * F1, DOUT], I8)
    grp0w = nc.dram_tensor("grp0w", [DIN, F1], I8)
    grp1w = nc.dram_tensor("grp1w", [F1, DOUT], I8)

    g0_in = nc.dram_tensor("g0_in", [H, NSH, C0], BF16)
    g0_out = nc.dram_tensor("g0_out", [NCORES, H, NSH, C0], BF16)
    g0s_in = nc.dram_tensor("g0s_in", [H, NSH, 2], F32)
    g0s_out = nc.dram_tensor("g0s_out", [NCORES, H, NSH, 2], F32)
    g1_in = nc.dram_tensor("g1_in", [H, NSH, C1], BF16)
    g1_out = nc.dram_tensor("g1_out", [NCORES, H, NSH, C1], BF16)
    g1s_in = nc.dram_tensor("g1s_in", [H, NSH, 2], F32)
    g1s_out = nc.dram_tensor("g1s_out", [NCORES, H, NSH, 2], F32)

    groups = [list(range(NCORES))]

    with tile.TileContext(nc) as tc:
        with (
            tc.tile_pool(name="persist", bufs=1) as pp,
            tc.tile_pool(name="sb", bufs=2) as sb,
            tc.tile_pool(name="small", bufs=3) as sm,
        ):
            ident = pp.tile([P, P], F32)
            masks.make_identity(nc, ident[:])
            identb = pp.tile([P, P], BF16)
            nc.vector.tensor_copy(identb[:], ident[:])
            h2pre = pp.tile([P, NIB, DOUT], F32)

            # param shards -> full weights, gathered on-device.
            # (collectives cannot read IO tensors; stage via internal DRAM)
            nc.sync.dma_start(out=W0i[:, :],
                              in_=qblob[0, QW0_OFF:QW0_OFF + W0_SZ])
            nc.sync.dma_start(out=rp0wi[:, :],
                              in_=qblob[0, QRP0_OFF:QRP0_OFF + RP0_SZ])
            nc.sync.dma_start(out=W1i[:, :],
                              in_=qblob[0, QW1_OFF:QW1_OFF + W1_SZ])
            nc.sync.dma_start(out=rp1wi[:, :],
                              in_=qblob[0, QRP1_OFF:QRP1_OFF + RP1_SZ])
            nc.gpsimd.collective_compute(
                "AllGather", alu.bypass, replica_groups=groups,
                ins=[W0i[:, :].opt()], outs=[gW0[:, :].opt()])
            nc.gpsimd.collective_compute(
                "AllGather", alu.bypass, replica_groups=groups,
                ins=[rp0wi[:, :].opt()], outs=[grp0w[:, :].opt()])
            nc.gpsimd.collective_compute(
                "AllGather", alu.bypass, replica_groups=groups,
                ins=[W1i[:, :].opt()], outs=[gW1[:, :].opt()])
            nc.gpsimd.collective_compute(
                "AllGather", alu.bypass, replica_groups=groups,
                ins=[rp1wi[:, :].opt()], outs=[grp1w[:, :].opt()])

            def bcast(pool, dram_row, width, name):
                row = pool.tile([1, width], F32, tag="bc_row", bufs=1,
                                name=f"r_{name}")
                nc.sync.dma_start(out=row[:], in_=dram_row)
                out = pool.tile([P, width], F32, name=f"b_{name}")
                nc.gpsimd.partition_broadcast(out[:], row[0:1, :])
                return out

            def ln_elu(pool, x_ap, gb, bb, width, out_ap, do_elu):
                """LN over free dim; x_ap is clobbered as scratch (B0)."""
                b1 = pool.tile([P, width], F32, tag="ln_b1", bufs=1,
                               name="ln_b1")
                b2 = pool.tile([P, width], F32, tag="ln_b2", bufs=1,
                               name="ln_b2")
                s1 = sm.tile([P, 1], F32, tag="ln_s1", name="ln_s1")
                nc.vector.tensor_reduce(s1[:], x_ap, mybir.AxisListType.X,
                                        alu.add)
                negmean = sm.tile([P, 1], F32, tag="ln_nm", name="ln_nm")
                nc.vector.tensor_single_scalar(negmean[:], s1[:],
                                               -1.0 / width, alu.mult)
                nc.scalar.activation(b1[:], x_ap, AF.Identity,
                                     bias=negmean[:, 0:1])          # t
                ssq = sm.tile([P, 1], F32, tag="ln_ssq", name="ln_ssq")
                nc.scalar.activation(b2[:], b1[:], AF.Square,
                                     accum_out=ssq[:, 0:1])
                var = sm.tile([P, 1], F32, tag="ln_var", name="ln_var")
                nc.vector.tensor_scalar(var[:], ssq[:], 1.0 / width, EPS,
                                        alu.mult, alu.add)
                std = sm.tile([P, 1], F32, tag="ln_std", name="ln_std")
                nc.scalar.activation(std[:], var[:], AF.Sqrt)
                rstd = sm.tile([P, 1], F32, tag="ln_rstd", name="ln_rstd")
                nc.vector.reciprocal(rstd[:], std[:])
                nc.scalar.mul(b2[:], b1[:], rstd[:, 0:1])           # u
                nc.vector.tensor_tensor(b1[:], b2[:], gb, alu.mult)  # v
                if not do_elu:
                    nc.vector.tensor_tensor(out_ap, b1[:], bb, alu.add)
                    return
                nc.vector.tensor_tensor(b2[:], b1[:], bb, alu.add)   # w
                nc.vector.tensor_single_scalar(b1[:], b2[:], 0.0, alu.min)
                nc.scalar.activation(x_ap, b1[:], AF.Exp)            # -> B0
                nc.vector.tensor_single_scalar(b1[:], b2[:], 0.0, alu.max)
                nc.vector.scalar_tensor_tensor(out_ap, x_ap, -1.0, b1[:],
                                               alu.add, alu.add)

            def attention(lid, O, N1, g_out, gs_out, gs_in, dest, mean_heads):
                CX = O + 2
                with (
                    tc.tile_pool(name=f"att{lid}", bufs=1) as ap_,
                    tc.tile_pool(name=f"att{lid}_d", bufs=3) as ad,
                    tc.tile_pool(name=f"att{lid}_ps", bufs=1,
                                 space="PSUM") as aps,
                ):
                    ssb = []
                    for h in range(H):
                        row = sm.tile([1, NSH], F32, tag="ssrow",
                                      name=f"ssrow{lid}_{h}")
                        nc.sync.dma_start(
                            out=row[:],
                            in_=gs_in[h, :, 0:1].rearrange("q c -> c q"))
                        sbh = ap_.tile([P, NSH], F32, name=f"ssb{lid}_{h}")
                        nc.gpsimd.partition_broadcast(sbh[:], row[0:1, :])
                        ssb.append(sbh)
                    acc = [ap_.tile([P, NIB, O + 1], F32,
                                    name=f"acc{lid}_{hh}") for hh in range(H)]
                    whs = ap_.tile([P, CH, H, CX], BF16)
                    svs = ap_.tile([P, CH, H, 2], F32)
                    ewsTv = ewsTd.rearrange("(c p) q -> p c q", p=P)
                    for jc in range(NCHUNK):
                        ewsc = ad.tile([P, CH, NSH], BF16, tag="ewsc",
                                       bufs=1, name="ewsc")
                        nc.sync.dma_start(
                            out=ewsc[:],
                            in_=ewsTv[:, jc * CH:(jc + 1) * CH, :])
                        ewpc = ad.tile([P, CH, NSH], BF16, tag="ewpc",
                                       bufs=1, name="ewpc")
                        nc.vector.tensor_scalar(
                            ewpc[:], ewsc[:], 0.0, 1.0 / QSCALE,
                            alu.max, alu.mult)
                        mtc = ad.tile([P, CH, NSH], BF16, tag="mtc",
                                      bufs=1, name="mtc")
                        nc.vector.tensor_single_scalar(
                            mtc[:], ewsc[:], 0.0, alu.is_ge)
                        for jt in range(CH):
                            jg = jc * CH + jt
                            s, r = jg // NIB, jg % NIB
                            nc.sync.dma_start(
                                out=whs[:, jt, :, :],
                                in_=g_out[s, :, r * P:(r + 1) * P, :]
                                .rearrange("h p c -> p h c"))
                            nc.sync.dma_start(
                                out=svs[:, jt, :, :],
                                in_=gs_out[s, :, r * P:(r + 1) * P, :]
                                .rearrange("h p c -> p h c"))
                        for h in range(H):
                            psa = [aps.tile([P, N1], F32, tag=f"psa{qb}",
                                            name=f"psa_{qb}")
                                   for qb in range(NIB)]
                            psb = [aps.tile([P, 257], F32, tag=f"psb{qb}",
                                            name=f"psb_{qb}")
                                   for qb in range(NIB)]
                            for jt in range(CH):
                                e = ad.tile([P, NSH], F32, tag="e", name="e")
                                nc.scalar.activation(
                                    e[:], ssb[h][:, :], AF.Lrelu,
                                    bias=svs[:, jt, h, 1:2], alpha=ALPHA)
                                att = ad.tile([P, NSH], F32, tag="att",
                                              name="att")
                                nc.vector.tensor_tensor(
                                    att[:], e[:], ewpc[:, jt, :], alu.mult)
                                pt = ad.tile([P, NSH], BF16, tag="pt",
                                             name="pt")
                                nc.scalar.activation(pt[:], att[:], AF.Exp)
                                ptm = ad.tile([P, NSH], BF16, tag="ptm",
                                              name="ptm")
                                nc.vector.tensor_tensor(
                                    ptm[:], pt[:], mtc[:, jt, :], alu.mult)
                                for qb in range(NIB):
                                    lhs = ptm[:, qb * P:(qb + 1) * P]
                                    nc.tensor.matmul(
                                        psa[qb][:], lhs, whs[:, jt, h, 0:N1],
                                        start=(jt == 0), stop=(jt == CH - 1))
                                    nc.tensor.matmul(
                                        psb[qb][:], lhs,
                                        whs[:, jt, h, N1:N1 + 257],
                                        start=(jt == 0), stop=(jt == CH - 1))
                            for qb in range(NIB):
                                if jc == 0:
                                    nc.vector.tensor_copy(
                                        acc[h][:, qb, 0:N1], psa[qb][:])
                                    nc.vector.tensor_copy(
                                        acc[h][:, qb, N1:O + 1], psb[qb][:])
                                else:
                                    nc.vector.scalar_tensor_tensor(
                                        acc[h][:, qb, 0:N1], psa[qb][:], 0.0,
                                        acc[h][:, qb, 0:N1], alu.add, alu.add)
                                    nc.vector.scalar_tensor_tensor(
                                        acc[h][:, qb, N1:O + 1], psb[qb][:],
                                        0.0, acc[h][:, qb, N1:O + 1],
                                        alu.add, alu.add)
                    for h in range(H):
                        for qb in range(NIB):
                            den = sm.tile([P, 1], F32, tag="den", name="den")
                            if mean_heads:
                                nc.vector.tensor_single_scalar(
                                    den[:], acc[h][:, qb, O:O + 1], float(H),
                                    alu.mult)
                            else:
                                nc.vector.tensor_copy(
                                    den[:], acc[h][:, qb, O:O + 1])
                            rcp = sm.tile([P, 1], F32, tag="rcp", name="rcp")
                            nc.vector.reciprocal(rcp[:], den[:])
                            out_ap = (dest[:, qb, 0:O] if mean_heads else
                                      dest[:, qb, h * O:(h + 1) * O])
                            nc.vector.scalar_tensor_tensor(
                                out_ap, acc[h][:, qb, 0:O], rcp[:, 0:1],
                                out_ap, alu.mult, alu.add)

            # ---- poolX: h1pre / h1 / h1T ----
            with tc.tile_pool(name="poolX", bufs=1) as px:
                # ===== prep: transpose ews on device -> ewsTd (DRAM) =====
                with (
                    tc.tile_pool(name="prep", bufs=2) as pr,
                    tc.tile_pool(name="prep_ps", bufs=2, space="PSUM") as prps,
                ):
                    ewsTw = ewsTd.rearrange("(jt p) q -> p jt q", p=P)
                    for qb in range(NIB):
                        ewq = pr.tile([P, N], BF16, tag="ewq", name="ewq")
                        nc.gpsimd.dma_start(
                            out=ewq[:],
                            in_=qblob[0, QEWS_OFF + qb * P * N:
                                      QEWS_OFF + (qb + 1) * P * N]
                            .rearrange("(p j) -> p j", p=P))
                        st = pr.tile([P, NJT, P], BF16, tag="ewst",
                                     name="ewst")
                        for jt in range(NJT):
                            pstb = prps.tile([P, P], BF16, tag="prtb",
                                             name="prtb")
                            nc.tensor.transpose(
                                pstb[:], ewq[:, jt * P:(jt + 1) * P],
                                identb[:])
                            nc.scalar.copy(st[:, jt, :], pstb[:])
                        nc.sync.dma_start(
                            out=ewsTw[:, :, qb * P:(qb + 1) * P],
                            in_=st[:])

                h1pre = px.tile([P, NIB, F1], F32)

                # ===== Phase A =====
                with (
                    tc.tile_pool(name="phA", bufs=1) as pa,
                    tc.tile_pool(name="phA_ps", bufs=2, space="PSUM") as paps,
                ):
                    sc0 = pa.tile([P, SC_W0 // P], F32, name="sc0")
                    nc.gpsimd.dma_start(
                        out=sc0[:],
                        in_=blob[0, SW0_OFF:SW0_OFF + SC_W0]
                        .rearrange("(t p) -> p t", p=P))
                    src0 = pa.tile([P, SC_RP0 // P], F32, name="src0")
                    nc.gpsimd.dma_start(
                        out=src0[:],
                        in_=blob[0, SRP0_OFF:SRP0_OFF + SC_RP0]
                        .rearrange("(t p) -> p t", p=P))
                    a0b = bcast(pa, sblob[:, SB_A0:SB_A0 + H * 2 * HID],
                                H * 2 * HID, "a0")
                    a0b = a0b.rearrange("p (h c) -> p h c", h=H)
                    rp0bb = bcast(pa, sblob[:, SB_RP0B:SB_RP0B + F1],
                                  F1, "rp0b")
                    s_sb0 = pa.tile([P, H, NIB, 2], F32)

                    # transpose nf on device -> nfTbf
                    nfTbf = pa.tile([P, DIN // P, NSH], BF16)
                    with (
                        tc.tile_pool(name="nfp", bufs=1) as npr,
                        tc.tile_pool(name="nfp_ps", bufs=2,
                                     space="PSUM") as nps,
                    ):
                        nfsb = npr.tile([P, NIB, DIN], BF16)
                        nc.sync.dma_start(
                            out=nfsb[:],
                            in_=blob[0, NF_OFF:NF_OFF + NF_SZ]
                            .rearrange("(b p f) -> p b f", p=P, f=DIN))
                        for qb in range(NIB):
                            for kb in range(DIN // P):
                                pst = nps.tile([P, P], BF16, tag="nft",
                                               name="nft")
                                nc.tensor.transpose(
                                    pst[:],
                                    nfsb[:, qb, kb * P:(kb + 1) * P],
                                    identb[:])
                                nc.scalar.copy(
                                    nfTbf[:, kb, qb * P:(qb + 1) * P],
                                    pst[:])

                    for h in range(H):
                        psv = [paps.tile([P, HID], F32, tag=f"wh0ps{ib}",
                                         bufs=1, name=f"wh0ps_{ib}")
                               for ib in range(NIB)]
                        for k in range(DIN // P):
                            w0r = sb.tile([P, HID], BF16, tag="w0r",
                                          bufs=3, name="w0r")
                            nc.gpsimd.dma_start(
                                out=w0r[:],
                                in_=gW0[h * DIN + k * P:h * DIN + (k + 1) * P,
                                        :])
                            w0t = sb.tile([P, HID], BF16, tag="w0t",
                                          bufs=3, name="w0t")
                            t0i = h * (DIN // P) + k
                            nc.scalar.mul(w0t[:], w0r[:],
                                          sc0[:, t0i:t0i + 1])
                            for ib in range(NIB):
                                nc.tensor.matmul(
                                    psv[ib][:],
                                    nfTbf[:, k, ib * P:(ib + 1) * P],
                                    w0t[:],
                                    start=(k == 0), stop=(k == DIN // P - 1))
                        for ib in range(NIB):
                            ps = psv[ib]
                            whtmp = sb.tile([P, HID], F32, tag="whtmp",
                                            bufs=1, name="whtmp")
                            nc.scalar.copy(whtmp[:], ps[:])
                            for which in range(2):
                                tmp = sb.tile([P, HID], F32, tag="sred",
                                              bufs=1, name="sred")
                                nc.vector.tensor_tensor(
                                    tmp[:], whtmp[:],
                                    a0b[:, h, which * HID:(which + 1) * HID],
                                    alu.mult)
                                nc.vector.tensor_reduce(
                                    s_sb0[:, h, ib, which:which + 1], tmp[:],
                                    mybir.AxisListType.X, alu.add)
                            pack = sb.tile([P, C0], BF16, tag="pack0",
                                           name="pack")
                            nc.vector.tensor_copy(pack[:, 0:HID], whtmp[:])
                            nc.vector.memset(pack[:, HID:HID + 1], 1.0)
                            nc.vector.memset(pack[:, HID + 1:C0], 0.0)
                            nc.sync.dma_start(
                                out=g0_in[h, ib * P:(ib + 1) * P, :],
                                in_=pack[:])
                    nc.sync.dma_start(
                        out=g0s_in.rearrange("h (ib p) c -> p h ib c", p=P),
                        in_=s_sb0[:])
                    nc.gpsimd.collective_compute(
                        "AllGather", alu.bypass, replica_groups=groups,
                        ins=[g0_in[:, :, :].opt()],
                        outs=[g0_out[:, :, :, :].opt()])
                    nc.gpsimd.collective_compute(
                        "AllGather", alu.bypass, replica_groups=groups,
                        ins=[g0s_in[:, :, :].opt()],
                        outs=[g0s_out[:, :, :, :].opt()])

                    rp0wsb = pa.tile([P, DIN // P, F1], BF16)
                    for k in range(DIN // P):
                        rp0raw = sb.tile([P, F1], BF16, tag="rp0raw",
                                         bufs=2, name="rp0raw")
                        nc.gpsimd.dma_start(
                            out=rp0raw[:],
                            in_=grp0w[k * P:(k + 1) * P, :])
                        nc.scalar.mul(rp0wsb[:, k, :], rp0raw[:],
                                      src0[:, k:k + 1])
                    for ib in range(NIB):
                        for oc in range(4):
                            ps2 = paps.tile([P, 512], F32, tag="rp0ps",
                                            name="ps2")
                            for k in range(DIN // P):
                                nc.tensor.matmul(
                                    ps2[:], nfTbf[:, k, ib * P:(ib + 1) * P],
                                    rp0wsb[:, k, oc * 512:(oc + 1) * 512],
                                    start=(k == 0), stop=(k == DIN // P - 1))
                            nc.vector.tensor_tensor(
                                h1pre[:, ib, oc * 512:(oc + 1) * 512],
                                ps2[:], rp0bb[:, oc * 512:(oc + 1) * 512],
                                alu.add)

                attention(0, HID, 256, g0_out, g0s_out, g0s_in, h1pre, False)

                h1T = px.tile([P, F1 // P, NSH], BF16)
                # ===== LN0 + ELU -> h1, transpose -> h1T =====
                with tc.tile_pool(name="ln0p", bufs=1) as lp0:
                    ln0gb = bcast(lp0, sblob[:, SB_LN0G:SB_LN0G + F1],
                                  F1, "ln0g")
                    ln0bb = bcast(lp0, sblob[:, SB_LN0B:SB_LN0B + F1],
                                  F1, "ln0b")
                    for ib in range(NIB):
                        ln_elu(lp0, h1pre[:, ib, :], ln0gb[:, :],
                               ln0bb[:, :], F1, h1pre[:, ib, :], True)
                with tc.tile_pool(name="trps", bufs=2, space="PSUM") as tps:
                    for ib in range(NIB):
                        for fb in range(F1 // P):
                            pst = tps.tile([P, P], F32, tag="pst",
                                           name="pst")
                            nc.tensor.transpose(
                                pst[:], h1pre[:, ib, fb * P:(fb + 1) * P],
                                ident[:])
                            nc.scalar.copy(
                                h1T[:, fb, ib * P:(ib + 1) * P], pst[:])

                # ===== Phase B =====
                with (
                    tc.tile_pool(name="phB", bufs=1) as pb,
                    tc.tile_pool(name="phB_d", bufs=3) as pbd,
                    tc.tile_pool(name="phB_ps", bufs=1, space="PSUM") as pbps,
                ):
                    sc1 = pb.tile([P, SC_W1 // P], F32, name="sc1")
                    nc.gpsimd.dma_start(
                        out=sc1[:],
                        in_=blob[0, SW1_OFF:SW1_OFF + SC_W1]
                        .rearrange("(t p) -> p t", p=P))
                    scr1 = pb.tile([P, SC_RP1 // P], F32, name="scr1")
                    nc.gpsimd.dma_start(
                        out=scr1[:],
                        in_=blob[0, SRP1_OFF:SRP1_OFF + SC_RP1]
                        .rearrange("(t p) -> p t", p=P))
                    a1bs = [bcast(pb, sblob[:, SB_A1 + hh * 2 * DOUT:
                                            SB_A1 + (hh + 1) * 2 * DOUT],
                                  2 * DOUT, f"a1_{hh}") for hh in range(H)]
                    rp1bb = bcast(pb, sblob[:, SB_RP1B:SB_RP1B + DOUT],
                                  DOUT, "rp1b")
                    s_sb1 = pb.tile([P, H, NIB, 2], F32)
                    halves = ((0, 512), (512, DOUT))
                    for h in range(H):
                        psw = [pbps.tile([P, DOUT], F32, tag=f"wh1ps{ib}",
                                         name=f"wh1ps_{ib}")
                               for ib in range(NIB)]
                        for k in range(F1 // P):
                            w1r = pbd.tile([P, DOUT], BF16, tag="w1r",
                                           name="w1r")
                            nc.gpsimd.dma_start(
                                out=w1r[:],
                                in_=gW1[h * F1 + k * P:h * F1 + (k + 1) * P,
                                        :])
                            w1t = pbd.tile([P, DOUT], BF16, tag="w1t",
                                           name="w1t")
                            t1i = h * (F1 // P) + k
                            nc.scalar.mul(w1t[:], w1r[:],
                                          sc1[:, t1i:t1i + 1])
                            for ib in range(NIB):
                                for (o0, o1) in halves:
                                    nc.tensor.matmul(
                                        psw[ib][:, o0:o1],
                                        h1T[:, k, ib * P:(ib + 1) * P],
                                        w1t[:, o0:o1],
                                        start=(k == 0),
                                        stop=(k == F1 // P - 1))
                        for ib in range(NIB):
                            whtmp1 = sb.tile([P, DOUT], F32, tag="whtmp1",
                                             bufs=1, name="whtmp1")
                            nc.scalar.copy(whtmp1[:], psw[ib][:])
                            for which in range(2):
                                tmp = sb.tile([P, DOUT], F32, tag="sred1",
                                              bufs=1, name="tmp")
                                nc.vector.tensor_tensor(
                                    tmp[:], whtmp1[:],
                                    a1bs[h][:, which * DOUT:(which + 1) * DOUT],
                                    alu.mult)
                                nc.vector.tensor_reduce(
                                    s_sb1[:, h, ib, which:which + 1], tmp[:],
                                    mybir.AxisListType.X, alu.add)
                            pack1 = sb.tile([P, C1], BF16, tag="pack1",
                                            name="pack1")
                            nc.vector.tensor_copy(pack1[:, 0:DOUT],
                                                  whtmp1[:])
                            nc.vector.memset(pack1[:, DOUT:DOUT + 1], 1.0)
                            nc.vector.memset(pack1[:, DOUT + 1:C1], 0.0)
                            nc.sync.dma_start(
                                out=g1_in[h, ib * P:(ib + 1) * P, :],
                                in_=pack1[:])
                    nc.sync.dma_start(
                        out=g1s_in.rearrange("h (ib p) c -> p h ib c", p=P),
                        in_=s_sb1[:])
                    nc.gpsimd.collective_compute(
                        "AllGather", alu.bypass, replica_groups=groups,
                        ins=[g1_in[:, :, :].opt()],
                        outs=[g1_out[:, :, :, :].opt()])
                    nc.gpsimd.collective_compute(
                        "AllGather", alu.bypass, replica_groups=groups,
                        ins=[g1s_in[:, :, :].opt()],
                        outs=[g1s_out[:, :, :, :].opt()])

                    psr = [pbps.tile([P, DOUT], F32, tag=f"wh1ps{ib}",
                                     name=f"rp1ps_{ib}")
                           for ib in range(NIB)]
                    for k in range(F1 // P):
                        r1r = pbd.tile([P, DOUT], BF16, tag="r1r",
                                       name="r1r")
                        nc.gpsimd.dma_start(
                            out=r1r[:], in_=grp1w[k * P:(k + 1) * P, :])
                        r1t = pbd.tile([P, DOUT], BF16, tag="r1t",
                                       name="r1t")
                        nc.scalar.mul(r1t[:], r1r[:], scr1[:, k:k + 1])
                        for ib in range(NIB):
                            for (o0, o1) in halves:
                                nc.tensor.matmul(
                                    psr[ib][:, o0:o1],
                                    h1T[:, k, ib * P:(ib + 1) * P],
                                    r1t[:, o0:o1],
                                    start=(k == 0), stop=(k == F1 // P - 1))
                    for ib in range(NIB):
                        nc.vector.tensor_tensor(
                            h2pre[:, ib, :], psr[ib][:], rp1bb[:, :],
                            alu.add)

            attention(1, DOUT, 512, g1_out, g1s_out, g1s_in, h2pre, True)

            # ===== LN1 -> h2 out =====
            with tc.tile_pool(name="ln1p", bufs=1) as lp1:
                ln1gb = bcast(lp1, sblob[:, SB_LN1G:SB_LN1G + DOUT],
                              DOUT, "ln1g")
                ln1bb = bcast(lp1, sblob[:, SB_LN1B:SB_LN1B + DOUT],
                              DOUT, "ln1b")
                for ib in range(NIB):
                    o = sb.tile([P, DOUT], BF16, tag="hout", name="o")
                    ln_elu(lp1, h2pre[:, ib, :], ln1gb[:, :], ln1bb[:, :],
                           DOUT, o[:], False)
                    nc.sync.dma_start(out=h2[ib * P:(ib + 1) * P, :],
                                      in_=o[:])

    nc.finalize()
    return nc


_NC_CACHE = None


def _get_nc():
    global _NC_CACHE
    if _NC_CACHE is None:
        _NC_CACHE = build_nc()
    return _NC_CACHE


_SCRATCH = {}


def _scratch(name, shape, dtype):
    buf = _SCRATCH.get(name)
    if buf is None or buf.shape != shape or buf.dtype != dtype:
        buf = np.empty(shape, dtype)
        _SCRATCH[name] = buf
    return buf


def build_in_maps(node_features, adjacency, edge_weights, W0, a0, W1, a1,
                  rp0_w, rp0_b, rp1_w, rp1_b, ln0_g, ln0_b, ln1_g, ln1_b):
    bf = ml_dtypes.bfloat16
    nf = np.asarray(node_features, np.float32).astype(bf)
    adj = np.asarray(adjacency)
    ew = np.asarray(edge_weights, np.float32)

    # q[i,j] = round(ew*127) on edges (incl. diagonal); -1 off edges
    # (0 if ew == 0 exactly, preserving the reference's exp(0)=1 quirk).
    conn = adj != 0
    np.fill_diagonal(conn, True)
    fbuf = _scratch("fbuf", (N, N), np.float32)
    np.multiply(ew, np.float32(QSCALE), out=fbuf)
    np.add(fbuf, np.float32(0.5), out=fbuf)
    q = np.where(conn, fbuf.astype(np.int8), -(ew != 0).astype(np.int8))

    def qrows(x, rows, width):
        x = np.asarray(x, np.float32).reshape(rows, width)
        s = np.abs(x).max(axis=1, keepdims=True) * np.float32(1.0 / QSCALE)
        s[s == 0] = 1.0
        qw = np.rint(x / s).astype(np.int8)
        return qw, s.reshape(-1).astype(bf)

    w0q, w0s = qrows(W0, H * DIN, HID)
    w1q, w1s = qrows(W1, H * F1, DOUT)
    rp0q, rp0s = qrows(rp0_w, DIN, F1)
    rp1q, rp1s = qrows(rp1_w, F1, DOUT)

    blob = _scratch("blob", (NCORES, BLOB_SZ), bf)
    qb8 = _scratch("qb8", (NCORES, QBLOB_SZ), np.int8)
    for c in range(NCORES):
        rows = slice(c * NSH, (c + 1) * NSH)
        blob[c, NF_OFF:NF_OFF + NF_SZ] = nf[rows].reshape(-1)
        blob[c, SW0_OFF:SW0_OFF + SC_W0] = w0s
        blob[c, SW1_OFF:SW1_OFF + SC_W1] = w1s
        blob[c, SRP0_OFF:SRP0_OFF + SC_RP0] = rp0s
        blob[c, SRP1_OFF:SRP1_OFF + SC_RP1] = rp1s
        qb8[c, QEWS_OFF:QEWS_OFF + EWS_SZ] = q[rows].reshape(-1)
        qb8[c, QW0_OFF:QW0_OFF + W0_SZ] = \
            w0q[c * W0R:(c + 1) * W0R].reshape(-1)
        qb8[c, QW1_OFF:QW1_OFF + W1_SZ] = \
            w1q[c * W1R:(c + 1) * W1R].reshape(-1)
        qb8[c, QRP0_OFF:QRP0_OFF + RP0_SZ] = \
            rp0q[c * RP0R:(c + 1) * RP0R].reshape(-1)
        qb8[c, QRP1_OFF:QRP1_OFF + RP1_SZ] = \
            rp1q[c * RP1R:(c + 1) * RP1R].reshape(-1)

    sblob = np.concatenate([
        np.asarray(a0, np.float32).reshape(-1),
        np.asarray(a1, np.float32).reshape(-1),
        np.asarray(rp0_b, np.float32).reshape(-1),
        np.asarray(rp1_b, np.float32).reshape(-1),
        np.asarray(ln0_g, np.float32).reshape(-1),
        np.asarray(ln0_b, np.float32).reshape(-1),
        np.asarray(ln1_g, np.float32).reshape(-1),
        np.asarray(ln1_b, np.float32).reshape(-1),
    ]).reshape(1, SBLOB_SZ)

    return [{"blob": blob[c:c + 1], "qblob": qb8[c:c + 1], "sblob": sblob}
            for c in range(NCORES)]


def kernel(**inputs):
    in_maps = build_in_maps(**inputs)
    nc = _get_nc()
    res = run_bass_kernel_spmd(nc, in_maps, list(range(NCORES)))
    return np.concatenate([res.results[c]["h2"] for c in range(NCORES)],
                          axis=0).astype(np.float32)


# revision 29
# speedup vs baseline: 1.4220x; 1.0426x over previous
"""GAT-style 2-layer knowledge-graph encoder on 8 trn2 NeuronCores.

Sharding: query rows, 512 per core. Scores are built transposed ([j, q]) so
the exp'd attention matrix is directly the matmul lhsT (no PE transposes).
The softmax denominator comes from a ones-column appended to the gathered
Wh payload (an extra matmul output column, no reduction pass). Wh for each
layer is computed on the owning shard and AllGathered on-device (bf16).

Host->device traffic is minimized (the wall clock is transfer-bound over
the axon tunnel, ~80 MB/s):
  - weights ship as 1/8 row-shards in one bf16 blob and are AllGathered
    on device (collectives can't read IO tensors, so shards are staged
    through internal DRAM first);
  - edge weights + mask are fused into ONE int8 array in natural row
    layout:  q[i,j] = round(127*ew) on edges (incl. self-loops), else
    -1 (or 0 when ew == 0 exactly).  A gpsimd casting DMA widens int8
    -> bf16; after a PE transpose the device derives
      ewp = max(q, 0)/127   (score multiplier) and
      mt  = (q >= 0)        (post-exp multiplier),
    which reproduces the reference's exp(0)=1 behaviour when ew == 0
    (uniform quantization tracks exp's absolute-argument sensitivity
    better than bf16's relative error);
  - node features ship as bf16 rows and are PE-transposed on device;
  - h2 returns as bf16 and is widened to f32 on the host.
"""

import numpy as np
import ml_dtypes

try:
    import jax
    jax.config.update("jax_compilation_cache_dir", "/tmp/jax_ccache")
    jax.config.update("jax_persistent_cache_min_compile_time_secs", 0)
    jax.config.update("jax_persistent_cache_min_entry_size_bytes", 0)
except Exception:
    pass

import concourse.bass as bass
import concourse.bacc as bacc
import concourse.mybir as mybir
from concourse import tile, masks
from concourse.bass_utils import run_bass_kernel_spmd
from concourse.alu_op_type import AluOpType as alu

BF16 = mybir.dt.bfloat16
F32 = mybir.dt.float32
I8 = mybir.dt.int8

P = 128
NCORES = 8
N = 4096
NSH = 512          # rows per core
H = 4
DIN = 768
HID = 512
F1 = 2048
DOUT = 768
C0 = 514           # 512 Wh + ones + pad  (bf16)
C1 = 770           # 768 Wh + ones + pad  (bf16)
ALPHA = 0.2
EPS = 1e-5
NIB = NSH // P     # 4 row-blocks per core
CH = 4             # j-tiles per chunk
NCHUNK = (N // P) // CH
NJT = N // P       # 32 j-tiles
AF = mybir.ActivationFunctionType

# parameter shard row counts (full rows / 8)
W0R = H * DIN // NCORES      # 384 rows of [., HID]
W1R = H * F1 // NCORES       # 1024 rows of [., DOUT]
RP0R = DIN // NCORES         # 96 rows of [., F1]
RP1R = F1 // NCORES          # 256 rows of [., DOUT]

# bf16 blob layout (element offsets): one transfer per core
NF_SZ = NSH * DIN
EWS_SZ = NSH * N
W0_SZ = W0R * HID
W1_SZ = W1R * DOUT
RP0_SZ = RP0R * F1
RP1_SZ = RP1R * DOUT
SC_W1 = H * F1            # 8192 per-row dequant scales for int8 W1
NF_OFF = 0
W0_OFF = NF_OFF + NF_SZ
RP0_OFF = W0_OFF + W0_SZ
RP1_OFF = RP0_OFF + RP0_SZ
SW1_OFF = RP1_OFF + RP1_SZ
BLOB_SZ = SW1_OFF + SC_W1
QEWS_OFF = 0
QW1_OFF = QEWS_OFF + EWS_SZ
QBLOB_SZ = QW1_OFF + W1_SZ
QSCALE = 127.0

# f32 smalls blob layout
SB_A0 = 0
SB_A1 = SB_A0 + H * 2 * HID
SB_RP0B = SB_A1 + H * 2 * DOUT
SB_RP1B = SB_RP0B + F1
SB_LN0G = SB_RP1B + DOUT
SB_LN0B = SB_LN0G + F1
SB_LN1G = SB_LN0B + F1
SB_LN1B = SB_LN1G + DOUT
SBLOB_SZ = SB_LN1B + DOUT


def build_nc():
    nc = bacc.Bacc(num_devices=NCORES)

    blob = nc.declare_dram_parameter("blob", [1, BLOB_SZ], BF16,
                                     isOutput=False)
    qblob = nc.declare_dram_parameter("qblob", [1, QBLOB_SZ], I8,
                                      isOutput=False)
    sblob = nc.declare_dram_parameter("sblob", [1, SBLOB_SZ], F32,
                                      isOutput=False)
    h2 = nc.declare_dram_parameter("h2", [NSH, DOUT], BF16, isOutput=True)

    ewsTd = nc.dram_tensor("ewsTd", [N, NSH], BF16)
    W0i = nc.dram_tensor("W0i", [W0R, HID], BF16)
    W1i = nc.dram_tensor("W1i", [W1R, DOUT], I8)
    rp0wi = nc.dram_tensor("rp0wi", [RP0R, F1], BF16)
    rp1wi = nc.dram_tensor("rp1wi", [RP1R, DOUT], BF16)
    gW0 = nc.dram_tensor("gW0", [H * DIN, HID], BF16)
    gW1 = nc.dram_tensor("gW1", [H * F1, DOUT], I8)
    grp0w = nc.dram_tensor("grp0w", [DIN, F1], BF16)
    grp1w = nc.dram_tensor("grp1w", [F1, DOUT], BF16)

    g0_in = nc.dram_tensor("g0_in", [H, NSH, C0], BF16)
    g0_out = nc.dram_tensor("g0_out", [NCORES, H, NSH, C0], BF16)
    g0s_in = nc.dram_tensor("g0s_in", [H, NSH, 2], F32)
    g0s_out = nc.dram_tensor("g0s_out", [NCORES, H, NSH, 2], F32)
    g1_in = nc.dram_tensor("g1_in", [H, NSH, C1], BF16)
    g1_out = nc.dram_tensor("g1_out", [NCORES, H, NSH, C1], BF16)
    g1s_in = nc.dram_tensor("g1s_in", [H, NSH, 2], F32)
    g1s_out = nc.dram_tensor("g1s_out", [NCORES, H, NSH, 2], F32)

    groups = [list(range(NCORES))]

    with tile.TileContext(nc) as tc:
        with (
            tc.tile_pool(name="persist", bufs=1) as pp,
            tc.tile_pool(name="sb", bufs=2) as sb,
            tc.tile_pool(name="small", bufs=3) as sm,
        ):
            ident = pp.tile([P, P], F32)
            masks.make_identity(nc, ident[:])
            identb = pp.tile([P, P], BF16)
            nc.vector.tensor_copy(identb[:], ident[:])
            h2pre = pp.tile([P, NIB, DOUT], F32)

            # param shards -> full weights, gathered on-device.
            # (collectives cannot read IO tensors; stage via internal DRAM)
            nc.sync.dma_start(out=W0i[:, :],
                              in_=blob[0, W0_OFF:W0_OFF + W0_SZ])
            nc.sync.dma_start(out=rp0wi[:, :],
                              in_=blob[0, RP0_OFF:RP0_OFF + RP0_SZ])
            nc.sync.dma_start(out=W1i[:, :],
                              in_=qblob[0, QW1_OFF:QW1_OFF + W1_SZ])
            nc.sync.dma_start(out=rp1wi[:, :],
                              in_=blob[0, RP1_OFF:RP1_OFF + RP1_SZ])
            nc.gpsimd.collective_compute(
                "AllGather", alu.bypass, replica_groups=groups,
                ins=[W0i[:, :].opt()], outs=[gW0[:, :].opt()])
            nc.gpsimd.collective_compute(
                "AllGather", alu.bypass, replica_groups=groups,
                ins=[rp0wi[:, :].opt()], outs=[grp0w[:, :].opt()])
            nc.gpsimd.collective_compute(
                "AllGather", alu.bypass, replica_groups=groups,
                ins=[W1i[:, :].opt()], outs=[gW1[:, :].opt()])
            nc.gpsimd.collective_compute(
                "AllGather", alu.bypass, replica_groups=groups,
                ins=[rp1wi[:, :].opt()], outs=[grp1w[:, :].opt()])

            def bcast(pool, dram_row, width, name):
                row = pool.tile([1, width], F32, tag="bc_row", bufs=1,
                                name=f"r_{name}")
                nc.sync.dma_start(out=row[:], in_=dram_row)
                out = pool.tile([P, width], F32, name=f"b_{name}")
                nc.gpsimd.partition_broadcast(out[:], row[0:1, :])
                return out

            def ln_elu(pool, x_ap, gb, bb, width, out_ap, do_elu):
                """LN over free dim; x_ap is clobbered as scratch (B0)."""
                b1 = pool.tile([P, width], F32, tag="ln_b1", bufs=1,
                               name="ln_b1")
                b2 = pool.tile([P, width], F32, tag="ln_b2", bufs=1,
                               name="ln_b2")
                s1 = sm.tile([P, 1], F32, tag="ln_s1", name="ln_s1")
                nc.vector.tensor_reduce(s1[:], x_ap, mybir.AxisListType.X,
                                        alu.add)
                negmean = sm.tile([P, 1], F32, tag="ln_nm", name="ln_nm")
                nc.vector.tensor_single_scalar(negmean[:], s1[:],
                                               -1.0 / width, alu.mult)
                nc.scalar.activation(b1[:], x_ap, AF.Identity,
                                     bias=negmean[:, 0:1])          # t
                ssq = sm.tile([P, 1], F32, tag="ln_ssq", name="ln_ssq")
                nc.scalar.activation(b2[:], b1[:], AF.Square,
                                     accum_out=ssq[:, 0:1])
                var = sm.tile([P, 1], F32, tag="ln_var", name="ln_var")
                nc.vector.tensor_scalar(var[:], ssq[:], 1.0 / width, EPS,
                                        alu.mult, alu.add)
                std = sm.tile([P, 1], F32, tag="ln_std", name="ln_std")
                nc.scalar.activation(std[:], var[:], AF.Sqrt)
                rstd = sm.tile([P, 1], F32, tag="ln_rstd", name="ln_rstd")
                nc.vector.reciprocal(rstd[:], std[:])
                nc.scalar.mul(b2[:], b1[:], rstd[:, 0:1])           # u
                nc.vector.tensor_tensor(b1[:], b2[:], gb, alu.mult)  # v
                if not do_elu:
                    nc.vector.tensor_tensor(out_ap, b1[:], bb, alu.add)
                    return
                nc.vector.tensor_tensor(b2[:], b1[:], bb, alu.add)   # w
                nc.vector.tensor_single_scalar(b1[:], b2[:], 0.0, alu.min)
                nc.scalar.activation(x_ap, b1[:], AF.Exp)            # -> B0
                nc.vector.tensor_single_scalar(b1[:], b2[:], 0.0, alu.max)
                nc.vector.scalar_tensor_tensor(out_ap, x_ap, -1.0, b1[:],
                                               alu.add, alu.add)

            def attention(lid, O, N1, g_out, gs_out, gs_in, dest, mean_heads):
                CX = O + 2
                with (
                    tc.tile_pool(name=f"att{lid}", bufs=1) as ap_,
                    tc.tile_pool(name=f"att{lid}_d", bufs=3) as ad,
                    tc.tile_pool(name=f"att{lid}_ps", bufs=1,
                                 space="PSUM") as aps,
                ):
                    ssb = []
                    for h in range(H):
                        row = sm.tile([1, NSH], F32, tag="ssrow",
                                      name=f"ssrow{lid}_{h}")
                        nc.sync.dma_start(
                            out=row[:],
                            in_=gs_in[h, :, 0:1].rearrange("q c -> c q"))
                        sbh = ap_.tile([P, NSH], F32, name=f"ssb{lid}_{h}")
                        nc.gpsimd.partition_broadcast(sbh[:], row[0:1, :])
                        ssb.append(sbh)
                    acc = [ap_.tile([P, NIB, O + 1], F32,
                                    name=f"acc{lid}_{hh}") for hh in range(H)]
                    whs = ap_.tile([P, CH, H, CX], BF16)
                    svs = ap_.tile([P, CH, H, 2], F32)
                    ewsTv = ewsTd.rearrange("(c p) q -> p c q", p=P)
                    for jc in range(NCHUNK):
                        ewsc = ad.tile([P, CH, NSH], BF16, tag="ewsc",
                                       bufs=1, name="ewsc")
                        nc.sync.dma_start(
                            out=ewsc[:],
                            in_=ewsTv[:, jc * CH:(jc + 1) * CH, :])
                        ewpc = ad.tile([P, CH, NSH], BF16, tag="ewpc",
                                       bufs=1, name="ewpc")
                        nc.vector.tensor_scalar(
                            ewpc[:], ewsc[:], 0.0, 1.0 / QSCALE,
                            alu.max, alu.mult)
                        mtc = ad.tile([P, CH, NSH], BF16, tag="mtc",
                                      bufs=1, name="mtc")
                        nc.vector.tensor_single_scalar(
                            mtc[:], ewsc[:], 0.0, alu.is_ge)
                        for jt in range(CH):
                            jg = jc * CH + jt
                            s, r = jg // NIB, jg % NIB
                            nc.sync.dma_start(
                                out=whs[:, jt, :, :],
                                in_=g_out[s, :, r * P:(r + 1) * P, :]
                                .rearrange("h p c -> p h c"))
                            nc.sync.dma_start(
                                out=svs[:, jt, :, :],
                                in_=gs_out[s, :, r * P:(r + 1) * P, :]
                                .rearrange("h p c -> p h c"))
                        for h in range(H):
                            psa = [aps.tile([P, N1], F32, tag=f"psa{qb}",
                                            name=f"psa_{qb}")
                                   for qb in range(NIB)]
                            psb = [aps.tile([P, 257], F32, tag=f"psb{qb}",
                                            name=f"psb_{qb}")
                                   for qb in range(NIB)]
                            for jt in range(CH):
                                e = ad.tile([P, NSH], F32, tag="e", name="e")
                                nc.scalar.activation(
                                    e[:], ssb[h][:, :], AF.Lrelu,
                                    bias=svs[:, jt, h, 1:2], alpha=ALPHA)
                                att = ad.tile([P, NSH], F32, tag="att",
                                              name="att")
                                nc.vector.tensor_tensor(
                                    att[:], e[:], ewpc[:, jt, :], alu.mult)
                                pt = ad.tile([P, NSH], BF16, tag="pt",
                                             name="pt")
                                nc.scalar.activation(pt[:], att[:], AF.Exp)
                                ptm = ad.tile([P, NSH], BF16, tag="ptm",
                                              name="ptm")
                                nc.vector.tensor_tensor(
                                    ptm[:], pt[:], mtc[:, jt, :], alu.mult)
                                for qb in range(NIB):
                                    lhs = ptm[:, qb * P:(qb + 1) * P]
                                    nc.tensor.matmul(
                                        psa[qb][:], lhs, whs[:, jt, h, 0:N1],
                                        start=(jt == 0), stop=(jt == CH - 1))
                                    nc.tensor.matmul(
                                        psb[qb][:], lhs,
                                        whs[:, jt, h, N1:N1 + 257],
                                        start=(jt == 0), stop=(jt == CH - 1))
                            for qb in range(NIB):
                                if jc == 0:
                                    nc.vector.tensor_copy(
                                        acc[h][:, qb, 0:N1], psa[qb][:])
                                    nc.vector.tensor_copy(
                                        acc[h][:, qb, N1:O + 1], psb[qb][:])
                                else:
                                    nc.vector.scalar_tensor_tensor(
                                        acc[h][:, qb, 0:N1], psa[qb][:], 0.0,
                                        acc[h][:, qb, 0:N1], alu.add, alu.add)
                                    nc.vector.scalar_tensor_tensor(
                                        acc[h][:, qb, N1:O + 1], psb[qb][:],
                                        0.0, acc[h][:, qb, N1:O + 1],
                                        alu.add, alu.add)
                    for h in range(H):
                        for qb in range(NIB):
                            den = sm.tile([P, 1], F32, tag="den", name="den")
                            if mean_heads:
                                nc.vector.tensor_single_scalar(
                                    den[:], acc[h][:, qb, O:O + 1], float(H),
                                    alu.mult)
                            else:
                                nc.vector.tensor_copy(
                                    den[:], acc[h][:, qb, O:O + 1])
                            rcp = sm.tile([P, 1], F32, tag="rcp", name="rcp")
                            nc.vector.reciprocal(rcp[:], den[:])
                            out_ap = (dest[:, qb, 0:O] if mean_heads else
                                      dest[:, qb, h * O:(h + 1) * O])
                            nc.vector.scalar_tensor_tensor(
                                out_ap, acc[h][:, qb, 0:O], rcp[:, 0:1],
                                out_ap, alu.mult, alu.add)

            # ---- poolX: h1pre / h1 / h1T ----
            with tc.tile_pool(name="poolX", bufs=1) as px:
                # ===== prep: transpose ews on device -> ewsTd (DRAM) =====
                with (
                    tc.tile_pool(name="prep", bufs=2) as pr,
                    tc.tile_pool(name="prep_ps", bufs=2, space="PSUM") as prps,
                ):
                    ewsTw = ewsTd.rearrange("(jt p) q -> p jt q", p=P)
                    for qb in range(NIB):
                        ewq = pr.tile([P, N], BF16, tag="ewq", name="ewq")
                        nc.gpsimd.dma_start(
                            out=ewq[:],
                            in_=qblob[0, qb * P * N:(qb + 1) * P * N]
                            .rearrange("(p j) -> p j", p=P))
                        st = pr.tile([P, NJT, P], BF16, tag="ewst",
                                     name="ewst")
                        for jt in range(NJT):
                            pstb = prps.tile([P, P], BF16, tag="prtb",
                                             name="prtb")
                            nc.tensor.transpose(
                                pstb[:], ewq[:, jt * P:(jt + 1) * P],
                                identb[:])
                            nc.scalar.copy(st[:, jt, :], pstb[:])
                        nc.sync.dma_start(
                            out=ewsTw[:, :, qb * P:(qb + 1) * P],
                            in_=st[:])

                h1pre = px.tile([P, NIB, F1], F32)

                # ===== Phase A =====
                with (
                    tc.tile_pool(name="phA", bufs=1) as pa,
                    tc.tile_pool(name="phA_ps", bufs=2, space="PSUM") as paps,
                ):
                    a0b = bcast(pa, sblob[:, SB_A0:SB_A0 + H * 2 * HID],
                                H * 2 * HID, "a0")
                    a0b = a0b.rearrange("p (h c) -> p h c", h=H)
                    rp0bb = bcast(pa, sblob[:, SB_RP0B:SB_RP0B + F1],
                                  F1, "rp0b")
                    s_sb0 = pa.tile([P, H, NIB, 2], F32)

                    # transpose nf on device -> nfTbf
                    nfTbf = pa.tile([P, DIN // P, NSH], BF16)
                    with (
                        tc.tile_pool(name="nfp", bufs=1) as npr,
                        tc.tile_pool(name="nfp_ps", bufs=2,
                                     space="PSUM") as nps,
                    ):
                        nfsb = npr.tile([P, NIB, DIN], BF16)
                        nc.sync.dma_start(
                            out=nfsb[:],
                            in_=blob[0, NF_OFF:NF_OFF + NF_SZ]
                            .rearrange("(b p f) -> p b f", p=P, f=DIN))
                        for qb in range(NIB):
                            for kb in range(DIN // P):
                                pst = nps.tile([P, P], BF16, tag="nft",
                                               name="nft")
                                nc.tensor.transpose(
                                    pst[:],
                                    nfsb[:, qb, kb * P:(kb + 1) * P],
                                    identb[:])
                                nc.scalar.copy(
                                    nfTbf[:, kb, qb * P:(qb + 1) * P],
                                    pst[:])

                    for h in range(H):
                        psv = [paps.tile([P, HID], F32, tag=f"wh0ps{ib}",
                                         bufs=1, name=f"wh0ps_{ib}")
                               for ib in range(NIB)]
                        for k in range(DIN // P):
                            w0t = sb.tile([P, HID], BF16, tag="w0t",
                                          bufs=3, name="w0t")
                            nc.sync.dma_start(
                                out=w0t[:],
                                in_=gW0[h * DIN + k * P:h * DIN + (k + 1) * P,
                                        :])
                            for ib in range(NIB):
                                nc.tensor.matmul(
                                    psv[ib][:],
                                    nfTbf[:, k, ib * P:(ib + 1) * P],
                                    w0t[:],
                                    start=(k == 0), stop=(k == DIN // P - 1))
                        for ib in range(NIB):
                            ps = psv[ib]
                            whtmp = sb.tile([P, HID], F32, tag="whtmp",
                                            bufs=1, name="whtmp")
                            nc.scalar.copy(whtmp[:], ps[:])
                            for which in range(2):
                                tmp = sb.tile([P, HID], F32, tag="sred",
                                              bufs=1, name="sred")
                                nc.vector.tensor_tensor(
                                    tmp[:], whtmp[:],
                                    a0b[:, h, which * HID:(which + 1) * HID],
                                    alu.mult)
                                nc.vector.tensor_reduce(
                                    s_sb0[:, h, ib, which:which + 1], tmp[:],
                                    mybir.AxisListType.X, alu.add)
                            pack = sb.tile([P, C0], BF16, tag="pack0",
                                           name="pack")
                            nc.vector.tensor_copy(pack[:, 0:HID], whtmp[:])
                            nc.vector.memset(pack[:, HID:HID + 1], 1.0)
                            nc.vector.memset(pack[:, HID + 1:C0], 0.0)
                            nc.sync.dma_start(
                                out=g0_in[h, ib * P:(ib + 1) * P, :],
                                in_=pack[:])
                    nc.sync.dma_start(
                        out=g0s_in.rearrange("h (ib p) c -> p h ib c", p=P),
                        in_=s_sb0[:])
                    nc.gpsimd.collective_compute(
                        "AllGather", alu.bypass, replica_groups=groups,
                        ins=[g0_in[:, :, :].opt()],
                        outs=[g0_out[:, :, :, :].opt()])
                    nc.gpsimd.collective_compute(
                        "AllGather", alu.bypass, replica_groups=groups,
                        ins=[g0s_in[:, :, :].opt()],
                        outs=[g0s_out[:, :, :, :].opt()])

                    rp0wsb = pa.tile([P, DIN // P, F1], BF16)
                    nc.sync.dma_start(
                        out=rp0wsb[:],
                        in_=grp0w.rearrange("(k p) o -> p k o", p=P))
                    for ib in range(NIB):
                        for oc in range(4):
                            ps2 = paps.tile([P, 512], F32, tag="rp0ps",
                                            name="ps2")
                            for k in range(DIN // P):
                                nc.tensor.matmul(
                                    ps2[:], nfTbf[:, k, ib * P:(ib + 1) * P],
                                    rp0wsb[:, k, oc * 512:(oc + 1) * 512],
                                    start=(k == 0), stop=(k == DIN // P - 1))
                            nc.vector.tensor_tensor(
                                h1pre[:, ib, oc * 512:(oc + 1) * 512],
                                ps2[:], rp0bb[:, oc * 512:(oc + 1) * 512],
                                alu.add)

                attention(0, HID, 256, g0_out, g0s_out, g0s_in, h1pre, False)

                h1T = px.tile([P, F1 // P, NSH], BF16)
                # ===== LN0 + ELU -> h1, transpose -> h1T =====
                with tc.tile_pool(name="ln0p", bufs=1) as lp0:
                    ln0gb = bcast(lp0, sblob[:, SB_LN0G:SB_LN0G + F1],
                                  F1, "ln0g")
                    ln0bb = bcast(lp0, sblob[:, SB_LN0B:SB_LN0B + F1],
                                  F1, "ln0b")
                    for ib in range(NIB):
                        ln_elu(lp0, h1pre[:, ib, :], ln0gb[:, :],
                               ln0bb[:, :], F1, h1pre[:, ib, :], True)
                with tc.tile_pool(name="trps", bufs=2, space="PSUM") as tps:
                    for ib in range(NIB):
                        for fb in range(F1 // P):
                            pst = tps.tile([P, P], F32, tag="pst",
                                           name="pst")
                            nc.tensor.transpose(
                                pst[:], h1pre[:, ib, fb * P:(fb + 1) * P],
                                ident[:])
                            nc.scalar.copy(
                                h1T[:, fb, ib * P:(ib + 1) * P], pst[:])

                # ===== Phase B =====
                with (
                    tc.tile_pool(name="phB", bufs=1) as pb,
                    tc.tile_pool(name="phB_d", bufs=3) as pbd,
                    tc.tile_pool(name="phB_ps", bufs=1, space="PSUM") as pbps,
                ):
                    sc1 = pb.tile([P, SC_W1 // P], F32, name="sc1")
                    nc.gpsimd.dma_start(
                        out=sc1[:],
                        in_=blob[0, SW1_OFF:SW1_OFF + SC_W1]
                        .rearrange("(t p) -> p t", p=P))
                    a1bs = [bcast(pb, sblob[:, SB_A1 + hh * 2 * DOUT:
                                            SB_A1 + (hh + 1) * 2 * DOUT],
                                  2 * DOUT, f"a1_{hh}") for hh in range(H)]
                    rp1bb = bcast(pb, sblob[:, SB_RP1B:SB_RP1B + DOUT],
                                  DOUT, "rp1b")
                    s_sb1 = pb.tile([P, H, NIB, 2], F32)
                    halves = ((0, 512), (512, DOUT))
                    for h in range(H):
                        psw = [pbps.tile([P, DOUT], F32, tag=f"wh1ps{ib}",
                                         name=f"wh1ps_{ib}")
                               for ib in range(NIB)]
                        for k in range(F1 // P):
                            w1r = pbd.tile([P, DOUT], BF16, tag="w1r",
                                           name="w1r")
                            nc.gpsimd.dma_start(
                                out=w1r[:],
                                in_=gW1[h * F1 + k * P:h * F1 + (k + 1) * P,
                                        :])
                            w1t = pbd.tile([P, DOUT], BF16, tag="w1t",
                                           name="w1t")
                            t1i = h * (F1 // P) + k
                            nc.scalar.mul(w1t[:], w1r[:],
                                          sc1[:, t1i:t1i + 1])
                            for ib in range(NIB):
                                for (o0, o1) in halves:
                                    nc.tensor.matmul(
                                        psw[ib][:, o0:o1],
                                        h1T[:, k, ib * P:(ib + 1) * P],
                                        w1t[:, o0:o1],
                                        start=(k == 0),
                                        stop=(k == F1 // P - 1))
                        for ib in range(NIB):
                            whtmp1 = sb.tile([P, DOUT], F32, tag="whtmp1",
                                             bufs=1, name="whtmp1")
                            nc.scalar.copy(whtmp1[:], psw[ib][:])
                            for which in range(2):
                                tmp = sb.tile([P, DOUT], F32, tag="sred1",
                                              bufs=1, name="tmp")
                                nc.vector.tensor_tensor(
                                    tmp[:], whtmp1[:],
                                    a1bs[h][:, which * DOUT:(which + 1) * DOUT],
                                    alu.mult)
                                nc.vector.tensor_reduce(
                                    s_sb1[:, h, ib, which:which + 1], tmp[:],
                                    mybir.AxisListType.X, alu.add)
                            pack1 = sb.tile([P, C1], BF16, tag="pack1",
                                            name="pack1")
                            nc.vector.tensor_copy(pack1[:, 0:DOUT],
                                                  whtmp1[:])
                            nc.vector.memset(pack1[:, DOUT:DOUT + 1], 1.0)
                            nc.vector.memset(pack1[:, DOUT + 1:C1], 0.0)
                            nc.sync.dma_start(
                                out=g1_in[h, ib * P:(ib + 1) * P, :],
                                in_=pack1[:])
                    nc.sync.dma_start(
                        out=g1s_in.rearrange("h (ib p) c -> p h ib c", p=P),
                        in_=s_sb1[:])
                    nc.gpsimd.collective_compute(
                        "AllGather", alu.bypass, replica_groups=groups,
                        ins=[g1_in[:, :, :].opt()],
                        outs=[g1_out[:, :, :, :].opt()])
                    nc.gpsimd.collective_compute(
                        "AllGather", alu.bypass, replica_groups=groups,
                        ins=[g1s_in[:, :, :].opt()],
                        outs=[g1s_out[:, :, :, :].opt()])

                    psr = [pbps.tile([P, DOUT], F32, tag=f"wh1ps{ib}",
                                     name=f"rp1ps_{ib}")
                           for ib in range(NIB)]
                    for k in range(F1 // P):
                        r1t = pbd.tile([P, DOUT], BF16, tag="r1t",
                                       name="r1t")
                        nc.sync.dma_start(
                            out=r1t[:], in_=grp1w[k * P:(k + 1) * P, :])
                        for ib in range(NIB):
                            for (o0, o1) in halves:
                                nc.tensor.matmul(
                                    psr[ib][:, o0:o1],
                                    h1T[:, k, ib * P:(ib + 1) * P],
                                    r1t[:, o0:o1],
                                    start=(k == 0), stop=(k == F1 // P - 1))
                    for ib in range(NIB):
                        nc.vector.tensor_tensor(
                            h2pre[:, ib, :], psr[ib][:], rp1bb[:, :],
                            alu.add)

            attention(1, DOUT, 512, g1_out, g1s_out, g1s_in, h2pre, True)

            # ===== LN1 -> h2 out =====
            with tc.tile_pool(name="ln1p", bufs=1) as lp1:
                ln1gb = bcast(lp1, sblob[:, SB_LN1G:SB_LN1G + DOUT],
                              DOUT, "ln1g")
                ln1bb = bcast(lp1, sblob[:, SB_LN1B:SB_LN1B + DOUT],
                              DOUT, "ln1b")
                for ib in range(NIB):
                    o = sb.tile([P, DOUT], BF16, tag="hout", name="o")
                    ln_elu(lp1, h2pre[:, ib, :], ln1gb[:, :], ln1bb[:, :],
                           DOUT, o[:], False)
                    nc.sync.dma_start(out=h2[ib * P:(ib + 1) * P, :],
                                      in_=o[:])

    nc.finalize()
    return nc


_NC_CACHE = None


def _get_nc():
    global _NC_CACHE
    if _NC_CACHE is None:
        _NC_CACHE = build_nc()
    return _NC_CACHE


_SCRATCH = {}


def _scratch(name, shape, dtype):
    buf = _SCRATCH.get(name)
    if buf is None or buf.shape != shape or buf.dtype != dtype:
        buf = np.empty(shape, dtype)
        _SCRATCH[name] = buf
    return buf


def build_in_maps(node_features, adjacency, edge_weights, W0, a0, W1, a1,
                  rp0_w, rp0_b, rp1_w, rp1_b, ln0_g, ln0_b, ln1_g, ln1_b):
    bf = ml_dtypes.bfloat16
    nf = np.asarray(node_features, np.float32).astype(bf)
    adj = np.asarray(adjacency)
    ew = np.asarray(edge_weights, np.float32)

    # q[i,j] = round(ew*127) on edges (incl. diagonal); -1 off edges
    # (0 if ew == 0 exactly, preserving the reference's exp(0)=1 quirk).
    conn = adj != 0
    np.fill_diagonal(conn, True)
    fbuf = _scratch("fbuf", (N, N), np.float32)
    np.multiply(ew, np.float32(QSCALE), out=fbuf)
    np.add(fbuf, np.float32(0.5), out=fbuf)
    q = np.where(conn, fbuf.astype(np.int8), -(ew != 0).astype(np.int8))

    w0 = np.asarray(W0, np.float32).reshape(H * DIN, HID).astype(bf)
    rp0w = np.asarray(rp0_w, np.float32).astype(bf)
    rp1w = np.asarray(rp1_w, np.float32).astype(bf)
    w1f = np.asarray(W1, np.float32).reshape(H * F1, DOUT)
    w1sc = np.abs(w1f).max(axis=1, keepdims=True) * np.float32(1.0 / QSCALE)
    w1sc[w1sc == 0] = 1.0
    w1q = np.rint(w1f / w1sc).astype(np.int8)
    w1s = w1sc.reshape(-1).astype(bf)

    blob = _scratch("blob", (NCORES, BLOB_SZ), bf)
    qb8 = _scratch("qb8", (NCORES, 1, QBLOB_SZ), np.int8)
    for c in range(NCORES):
        rows = slice(c * NSH, (c + 1) * NSH)
        blob[c, NF_OFF:NF_OFF + NF_SZ] = nf[rows].reshape(-1)
        blob[c, W0_OFF:W0_OFF + W0_SZ] = \
            w0[c * W0R:(c + 1) * W0R].reshape(-1)
        blob[c, RP0_OFF:RP0_OFF + RP0_SZ] = \
            rp0w[c * RP0R:(c + 1) * RP0R].reshape(-1)
        blob[c, RP1_OFF:RP1_OFF + RP1_SZ] = \
            rp1w[c * RP1R:(c + 1) * RP1R].reshape(-1)
        blob[c, SW1_OFF:SW1_OFF + SC_W1] = w1s
        qb8[c, 0, QEWS_OFF:QEWS_OFF + EWS_SZ] = q[rows].reshape(-1)
        qb8[c, 0, QW1_OFF:QW1_OFF + W1_SZ] = \
            w1q[c * W1R:(c + 1) * W1R].reshape(-1)

    sblob = np.concatenate([
        np.asarray(a0, np.float32).reshape(-1),
        np.asarray(a1, np.float32).reshape(-1),
        np.asarray(rp0_b, np.float32).reshape(-1),
        np.asarray(rp1_b, np.float32).reshape(-1),
        np.asarray(ln0_g, np.float32).reshape(-1),
        np.asarray(ln0_b, np.float32).reshape(-1),
        np.asarray(ln1_g, np.float32).reshape(-1),
        np.asarray(ln1_b, np.float32).reshape(-1),
    ]).reshape(1, SBLOB_SZ)

    return [{"blob": blob[c:c + 1], "qblob": qb8[c], "sblob": sblob}
            for c in range(NCORES)]


def kernel(**inputs):
    in_maps = build_in_maps(**inputs)
    nc = _get_nc()
    res = run_bass_kernel_spmd(nc, in_maps, list(range(NCORES)))
    return np.concatenate([res.results[c]["h2"] for c in range(NCORES)],
                          axis=0).astype(np.float32)
